# revision 15
# baseline (speedup 1.0000x reference)
"""Trainium2 Bass kernel for nn_MixtureOfAdapter (moe_routing).

Math (per token, H=1024, F=256, D=3 domains):
    mu, sd (ddof=1) over H;  s = sd + eps;  xn = (x - mu)/s
    h_d   = xn*g_d + b_d
    mid_d = relu(W1_d h_d + b1_d);  a_d = W2_d mid_d + b2_d
    gate_d = sigmoid(gu_d.x + gv_d.a_d + gb_d)
    out = 2x + sum_d gate_d * a_d

Kernel strategy (8 cores, data-parallel over batch B=8):
  - Both big GEMMs run as fp8e4 DoubleRow matmuls (0.5 cyc per moving
    row, 4x bf16 MACs/cycle; contraction = 128 partitions x 2 pair
    lanes).  Precision is held by hi+lo fp8 splitting: a value v is
    stored as v_hi = fp8(v), v_lo = fp8(v - v_hi), and the product
    (W_hi + W_lo)(x_hi + x_lo) is computed dropping only the lo*lo
    term.  The single remaining un-split tensor is gmid = fp8(mid *
    gate/16) (one ~3.6%-rms source -> ~1.3e-2 max rel err, tolerance
    2e-2; validated against the reference in numpy).
  - Weights are pre-scaled into fp8's normal range host-side:
    W1s = 32*W1*ln_g (psum descaled by Act relu scale=1/32) and
    W2s = 16*W2 (descale folded into the gate broadcast one-hot =
    1/16, so gmid = mid*gate/16 with no extra op).
  - M1: per df-chunk, 12 DR passes: W1hi@xn_hi, W1hi@xn_lo, W1lo@xn_hi
    (k-pair pairs).  Macro-tile 0 drops the W1lo term (8 passes) so the
    PE never stalls waiting for the w1lo DMA at startup; the slightly
    larger error on tokens 0..511 stays under the global bound.
  - xn is normalized to bf16, PE-transposed (1 cyc/row), then split
    psum -> (xnT_hi fp8 via Act copy, xnT_lo fp8 via DVE/Pool
    tensor_tensor subtract).
  - Gates: pgux via fp8 DR passes on (gus_hi, gus_lo); pgv = w2gv.mid
    in bf16; mu/s rank-1 corrections from a transposed (mu, s) pack;
    gate = sigmoid(pgux*(s/32) + pgv + gb_eff).
  - M2: per (sub-tile, h-chunk) 6 DR passes (3 f-pairs x {W2hi, W2lo})
    accumulate all domains; out = 2x + psum via DVE/Pool
    scalar_tensor_tensor.
  - Engine balance: the fp8 split/gmid elementwise work is spread
    across DVE, Act and Pool (gpsimd) so no engine exceeds the PE's
    ~15.8us/macro-tile.
  - DMA discipline (the TimelineSim serializes transfers on one
    DMA_ENGINES resource and charges ~630ns HWDGE descriptor gen per
    DMA): few large DMAs; startup loads x sub-tile 0 and the w1hi
    k-pair chunks first; w1lo follows; w2hi/w2lo are gated behind
    later x arrivals; steady x loads ride the SWDGE (Pool) queue paced
    by 1-element gating copies; outputs on SWDGE except the last
    macro-tile's, which go as halves on the idle sync queue.
  - Software-pipelined emission keeps each macro-tile's gate chain
    hidden behind the next tile's M1 in the PE FIFO; macro-tile 0's M1
    is emitted in 128-token column slices so the PE starts as soon as
    the first sub-tile's transpose lands.
  - _split_multiwaits rewrites >1-wait instructions (walrus limit)
    to park extra waits on Memset/Copy carrier ops.
"""

import numpy as np

import concourse.bass as bass
import concourse.mybir as mybir
import concourse.tile as tile
from concourse.bass_utils import run_bass_kernel_spmd

B, L, H, F, D = 8, 2048, 1024, 256, 3
EPS = 1e-6
T = 512                 # tokens per macro-tile
NSUB = T // 128         # 4 sub-tiles of 128 tokens
NMT = L // T            # 4 macro-tiles per core
KCH = H // 128          # 8 k-chunks over H
KP = KCH // 2           # 4 k-pair DR passes over H
FCH = (D * F) // 128    # 6 chunks over stacked (domain, F)
FP = FCH // 2           # 3 f-pair DR passes (one per domain)
NCH = H // 512          # 2 output column chunks
DF = D * F

W1SC = 32.0             # host pre-scale on W1*ln_g (and gu)
W2SC = 16.0             # host pre-scale on W2

f32 = mybir.dt.float32
bf16 = mybir.dt.bfloat16
f8 = mybir.dt.float8e4
AF = mybir.ActivationFunctionType
ALU = mybir.AluOpType
DR = mybir.MatmulPerfMode.DoubleRow

M1_TERMS = 3            # 2 = drop W1lo@xn_hi (faster, slightly less exact)
PGUX_TERMS = 3


def _split_multiwaits(nc):
    """This walrus build allows 1 sync-wait per instruction (2 for
    EventSemaphore); Tile can attach more.  Move extras onto preceding
    same-engine carrier instructions.  A bare NoOp holds the sequencer
    while it waits (stalling dispatch of everything behind it), so where
    possible the carrier is a 1-element Memset to a dead scratch column:
    a real engine instruction parks its wait in the engine wait queue
    and lets the sequencer keep dispatching."""
    import copy
    tmpl = {}
    for f in nc.m.functions:
        for bb in f.blocks:
            for inst in bb.instructions:
                if (isinstance(inst, mybir.InstMemset)
                        and inst.engine not in tmpl):
                    tmpl[inst.engine] = inst
                elif (isinstance(inst, mybir.InstActivation)
                        and inst.func == AF.Copy
                        and inst.engine not in tmpl):
                    tmpl[inst.engine] = inst

    def carrier(inst, w, j):
        t = tmpl.get(inst.engine)
        if t is not None:
            c = copy.deepcopy(t)
            c.name = f"{inst.name}-wsplit{j}"
            c.sync_info = mybir.SyncInfo(on_wait=[w], on_update=[])
            return c
        return mybir.InstNoOp(
            name=f"{inst.name}-wsplit{j}",
            engine=inst.engine,
            sync_info=mybir.SyncInfo(on_wait=[w], on_update=[]),
            ins=[], outs=[],
        )

    for f in nc.m.functions:
        for bb in f.blocks:
            new = []
            changed = False
            for inst in bb.instructions:
                si = inst.sync_info
                cap = 2 if isinstance(inst, mybir.InstEventSemaphore) else 1
                if si is not None and len(si.on_wait) > cap:
                    waits = list(si.on_wait)
                    extra, kept = waits[:-cap], waits[-cap:]
                    for j, w in enumerate(extra):
                        new.append(carrier(inst, w, j))
                    inst.sync_info = mybir.SyncInfo(
                        on_wait=kept, on_update=list(si.on_update))
                    changed = True
                new.append(inst)
            if changed:
                bb.instructions = new


def _build(has_b1e: bool, has_b2: bool):
    nc = bass.Bass(target_bir_lowering=False)

    xin = nc.dram_tensor("xin", [L, H], f32, kind="ExternalInput")
    w1hi = nc.dram_tensor("w1hi", [128, KP, 2, DF], f8, kind="ExternalInput")
    w1lo = nc.dram_tensor("w1lo", [128, KP, 2, DF], f8, kind="ExternalInput")
    w2hi = nc.dram_tensor("w2hi", [128, FCH, H], f8, kind="ExternalInput")
    w2lo = nc.dram_tensor("w2lo", [128, FCH, H], f8, kind="ExternalInput")
    gq = nc.dram_tensor("gq", [128, KCH, 32], f8, kind="ExternalInput")
    cpb = nc.dram_tensor("cpb", [128, 560], bf16, kind="ExternalInput")
    cpf = nc.dram_tensor("cpf", [128, 8], f32, kind="ExternalInput")
    if has_b2:
        b2r = nc.dram_tensor("b2r", [D, H], bf16, kind="ExternalInput")
    out = nc.dram_tensor("out", [L, H], f32, kind="ExternalOutput")

    # [L, H] viewed as [128p, sub, H] per macro-tile
    x_mt = xin.ap().rearrange("(m s p) h -> m p s h", p=128, s=NSUB)
    out_mt = out.ap().rearrange("(m s p) h -> m p s h", p=128, s=NSUB)

    with tile.TileContext(nc) as tc:
        with (
            tc.tile_pool(name="const", bufs=1) as const,
            tc.tile_pool(name="xp", bufs=3) as xp,
            tc.tile_pool(name="xnp", bufs=2) as xnp,
            tc.tile_pool(name="xtp", bufs=3) as xtp,
            tc.tile_pool(name="midp", bufs=3) as midp,
            tc.tile_pool(name="gmp", bufs=3) as gmp,
            tc.tile_pool(name="gbp", bufs=3) as gbp,
            tc.tile_pool(name="outp", bufs=4) as outp,
            tc.tile_pool(name="smalls", bufs=6) as smalls,
            tc.tile_pool(name="gsm", bufs=3) as gsm,
            tc.tile_pool(name="ps_m1", bufs=2, space="PSUM") as ps_m1,
            tc.tile_pool(name="ps_m2", bufs=2, space="PSUM") as ps_m2,
            tc.tile_pool(name="ps_gux", bufs=1, space="PSUM") as ps_gux,
            tc.tile_pool(name="ps_gv", bufs=1, space="PSUM") as ps_gv,
            tc.tile_pool(name="ps_tr", bufs=2, space="PSUM") as ps_tr,
        ):
            # scratch columns for multiwait carrier ops (dead stores)
            scratch = const.tile([128, 4], f32)
            nc.vector.memset(scratch[:, 0:1], 0.0)
            nc.gpsimd.memset(scratch[:, 2:3], 0.0)
            nc.scalar.copy(scratch[0:1, 1:2], scratch[0:1, 3:4])

            # constants on scalar queue (small, needed early)
            cpb_sb = const.tile([128, 560], bf16)
            cpf_sb = const.tile([128, 8], f32)
            gq_sb = const.tile([128, KCH, 32], f8)
            nc.scalar.dma_start(out=cpb_sb, in_=cpb.ap())
            nc.scalar.dma_start(out=cpf_sb, in_=cpf.ap())
            nc.scalar.dma_start(out=gq_sb, in_=gq.ap())
            oh_sb = cpb_sb[0:D, 0:384]              # one-hot rows = 1/16
            w2gv_sb = cpb_sb[:, 408:426].rearrange("p (c d) -> p c d", d=D)
            gusum_sb = cpb_sb[0:1, 426:429]
            ones3_sb = cpb_sb[32:33, 429:432]       # = 1/32
            ident_b = cpb_sb[:, 432:560]
            gb3_sb = cpf_sb[0:D, 6:7]
            # dual-fp8 ldweights needs pair stride >= 32: hi at cols
            # 0:3, lo at 16:19 of a 32-wide pack
            gqhi = gq_sb[:, :, 0:D]
            gqlo = gq_sb[:, :, 16:16 + D]

            # startup loads: x sub-tiles interleaved with w1hi halves so
            # the PE can start on sub-tile 0 as soon as possible
            x_first = xp.tile([128, NSUB, H], f32, tag="x")
            w1hi_sb = const.tile([128, KP, 2, DF], f8)
            w1lo_sb = const.tile([128, KP, 2, DF], f8)
            w2hi_sb = const.tile([128, FCH, H], f8)
            w2lo_sb = const.tile([128, FCH, H], f8)
            # x sub 0 in halves so bn_stats starts on the first half
            nc.sync.dma_start(out=x_first[:, 0, 0:512],
                              in_=x_mt[0][:, 0, 0:512])
            nc.sync.dma_start(out=x_first[:, 0, 512:1024],
                              in_=x_mt[0][:, 0, 512:1024])
            nc.sync.dma_start(
                out=w1hi_sb[:, 0:2].rearrange("p a b c -> p (a b c)"),
                in_=w1hi.ap()[:, 0:2].rearrange("p a b c -> p (a b c)"))
            nc.sync.dma_start(out=x_first[:, 1, :], in_=x_mt[0][:, 1, :])
            nc.sync.dma_start(
                out=w1hi_sb[:, 2:4].rearrange("p a b c -> p (a b c)"),
                in_=w1hi.ap()[:, 2:4].rearrange("p a b c -> p (a b c)"))
            nc.sync.dma_start(out=x_first[:, 2, :], in_=x_mt[0][:, 2, :])
            nc.sync.dma_start(out=x_first[:, 3, :], in_=x_mt[0][:, 3, :])
            nc.sync.dma_start(
                out=w1lo_sb.rearrange("p a b c -> p (a b c)"),
                in_=w1lo.ap().rearrange("p a b c -> p (a b c)"))
            if has_b2:
                b2r_sb = const.tile([D, H], bf16)
                nc.scalar.dma_start(out=b2r_sb, in_=b2r.ap())

            def stage_load(mt, x_pre=None, prev_x=None):
                """x load only (emitted early for DMA pacing)."""
                if x_pre is not None:
                    x_t = x_pre
                else:
                    x_t = xp.tile([128, NSUB, H], f32, tag="x")
                    if prev_x is not None:
                        nc.gpsimd.tensor_copy(x_t[0:1, 0, 0:1],
                                              prev_x[0:1, 2, 0:1])
                    nc.gpsimd.dma_start(out=x_t, in_=x_mt[mt])
                return x_t

            def stage_a(mt, x_t):
                """stats, normalize (bf16), PE transposes, fp8 hi/lo split."""
                xn_b = xnp.tile([128, NSUB, H], bf16, tag="xn")
                xnT_hi = xtp.tile([128, KCH, T], f8, tag="xnTh")
                xnT_lo = xtp.tile([128, KCH, T], f8, tag="xnTl")
                rows_b = xtp.tile([33, T], bf16, tag="rows")
                for ss in range(NSUB):
                    xs = x_t[:, ss, :]
                    tsl = slice(ss * 128, (ss + 1) * 128)
                    st = smalls.tile([128, 2, 6], f32, tag="bnst")
                    nc.vector.bn_stats(out=st[:, 0, :], in_=xs[:, 0:512])
                    nc.vector.bn_stats(out=st[:, 1, :], in_=xs[:, 512:1024])
                    mv = smalls.tile([128, 2], f32, tag="mv")
                    nc.vector.bn_aggr(out=mv, in_=st)
                    # sc: 0=r=1/s, 1=-mu*r, 2=s.
                    # s = sqrt(var*H/(H-1) + 2e-6) ~= sd + 1e-6 to ~5e-8.
                    sc = smalls.tile([128, 4], f32, tag="sc")
                    nc.scalar.activation(out=sc[:, 2:3], in_=mv[:, 1:2],
                                         func=AF.Sqrt,
                                         scale=float(H) / (H - 1),
                                         bias=cpf_sb[:, 7:8])
                    nc.vector.reciprocal(sc[:, 0:1], sc[:, 2:3])
                    nc.vector.tensor_scalar(out=sc[:, 1:2], in0=mv[:, 0:1],
                                            scalar1=sc[:, 0:1], scalar2=-1.0,
                                            op0=ALU.mult, op1=ALU.mult)
                    # (mu, s) pack for the row transpose
                    pk = smalls.tile([128, 33], bf16, tag="pk")
                    nc.vector.tensor_copy(pk[:, 0:1], mv[:, 0:1])
                    nc.vector.tensor_copy(pk[:, 32:33], sc[:, 2:3])
                    # xn = x*(1/s) + (-mu/s), bf16 out; alternate engines
                    if ss % 2 == 0:
                        nc.scalar.activation(out=xn_b[:, ss, :], in_=xs,
                                             func=AF.Identity,
                                             scale=sc[:, 0:1],
                                             bias=sc[:, 1:2])
                    else:
                        nc.gpsimd.tensor_scalar(out=xn_b[:, ss, :], in0=xs,
                                                scalar1=sc[:, 0:1],
                                                scalar2=sc[:, 1:2],
                                                op0=ALU.mult, op1=ALU.add)
                    # PE transposes (bf16: 1 cyc/row) into one full-bank
                    # [128, 1024] psum batch, then one wide fp8 hi copy
                    # (Act) and one wide lo subtract (DVE; gpsimd cannot
                    # read PSUM)
                    ptr = ps_tr.tile([128, 1024], bf16, tag="tr")
                    for k in range(KCH):
                        nc.tensor.transpose(
                            ptr[:, k * 128:(k + 1) * 128],
                            xn_b[:, ss, k * 128:(k + 1) * 128], ident_b)
                    dst_hi = xnT_hi[:, :, tsl]
                    dst_lo = xnT_lo[:, :, tsl]
                    src = ptr.rearrange("p (q t) -> p q t", q=KCH)
                    nc.scalar.activation(out=dst_hi, in_=src, func=AF.Copy)
                    nc.vector.tensor_tensor(out=dst_lo, in0=src, in1=dst_hi,
                                            op=ALU.subtract)
                    ptr2 = ps_tr.tile([33, 128], bf16, tag="tr")
                    nc.tensor.transpose(ptr2, pk, ident_b)
                    nc.scalar.activation(out=rows_b[:, tsl], in_=ptr2,
                                         func=AF.Copy)
                return dict(x_t=x_t, xnT_hi=xnT_hi, xnT_lo=xnT_lo,
                            rows=rows_b)

            def m1_passes(p1, c, xnT_hi, xnT_lo, tsl, terms):
                """Emit the DR passes for one df-chunk into psum p1."""
                cs = slice(c * 128, (c + 1) * 128)
                first = True
                seqs = [(w1hi_sb, xnT_hi), (w1hi_sb, xnT_lo)]
                if terms >= 3:
                    seqs.append((w1lo_sb, xnT_hi))
                n = len(seqs) * KP
                i = 0
                for w_sb, x_sb in seqs:
                    for kp in range(KP):
                        i += 1
                        nc.tensor.matmul(
                            p1[:, tsl] if tsl else p1,
                            w_sb[:, kp, :, cs],
                            x_sb[:, 2 * kp:2 * kp + 2, tsl]
                            if tsl else x_sb[:, 2 * kp:2 * kp + 2, :],
                            start=first, stop=(i == n), perf_mode=DR)
                        first = False

            def pgux_passes(pgux, xnT_hi, xnT_lo, tsl, terms, start):
                seqs = [(gqhi, xnT_hi), (gqhi, xnT_lo)]
                if terms >= 3:
                    seqs.append((gqlo, xnT_hi))
                n = len(seqs) * KP
                i = 0
                first = start
                for g_sb, x_sb in seqs:
                    for kp in range(KP):
                        i += 1
                        nc.tensor.matmul(
                            pgux[:, tsl] if tsl else pgux,
                            g_sb[:, 2 * kp:2 * kp + 2, :],
                            x_sb[:, 2 * kp:2 * kp + 2, tsl]
                            if tsl else x_sb[:, 2 * kp:2 * kp + 2, :],
                            start=first, stop=(i == n), perf_mode=DR)
                        first = False

            def stage_b(mt, st_, sliced=False):
                """M1: mid = relu((W1s @ xn)/32 (+ b1e)).  Sliced mode
                (macro-tile 0) runs token slices through four psums with
                pgux appended so all work for the first two sub-tiles is
                in the PE FIFO before anything waiting on later arrivals.
                mt0 also drops the W1lo term so the PE never waits on the
                w1lo DMA."""
                xnT_hi, xnT_lo = st_["xnT_hi"], st_["xnT_lo"]
                mid = midp.tile([128, FCH, T], bf16, tag="mid")
                terms = 2 if sliced else M1_TERMS
                if sliced:
                    p1s = []
                    for ci in range(2):
                        pw = ps_m1.tile([128, T], f32, tag="m1",
                                        name=f"m1w{ci}")
                        p1s.append(pw)
                    for ci in range(2):
                        pw = ps_m2.tile([128, 512], f32, tag="m2",
                                        name=f"m2w{ci}")
                        p1s.append(pw)
                    pgux = ps_gux.tile([D, T], f32, tag="gux")
                    st_["pgux"] = pgux
                    for ss in range(NSUB):
                        tsl = slice(ss * 128, (ss + 1) * 128)
                        for c in range(4):
                            m1_passes(p1s[c], c, xnT_hi, xnT_lo, tsl, terms)
                        if ss == 1:
                            for gss in range(2):
                                gsl = slice(gss * 128, (gss + 1) * 128)
                                pgux_passes(pgux, xnT_hi, xnT_lo, gsl, 2,
                                            start=True)
                    for c in range(4):
                        bias = cpf_sb[:, c:c + 1] if has_b1e else 0.0
                        nc.scalar.activation(out=mid[:, c, :], in_=p1s[c],
                                             func=AF.Relu, bias=bias,
                                             scale=1.0 / W1SC)
                    for c in range(4, FCH):
                        p1 = ps_m1.tile([128, T], f32, tag="m1")
                        for ss in range(NSUB):
                            tsl = slice(ss * 128, (ss + 1) * 128)
                            m1_passes(p1, c, xnT_hi, xnT_lo, tsl, terms)
                        bias = cpf_sb[:, c:c + 1] if has_b1e else 0.0
                        nc.scalar.activation(out=mid[:, c, :], in_=p1,
                                             func=AF.Relu, bias=bias,
                                             scale=1.0 / W1SC)
                    st_["mid"] = mid
                    return
                for c in range(FCH):
                    p1 = ps_m1.tile([128, T], f32, tag="m1")
                    m1_passes(p1, c, xnT_hi, xnT_lo, None, terms)
                    bias = cpf_sb[:, c:c + 1] if has_b1e else 0.0
                    nc.scalar.activation(out=mid[:, c, :], in_=p1,
                                         func=AF.Relu, bias=bias,
                                         scale=1.0 / W1SC)
                st_["mid"] = mid

            def stage_c(mt, st_):
                """Gates + g_t."""
                xnT_hi, xnT_lo, mid = (st_["xnT_hi"], st_["xnT_lo"],
                                       st_["mid"])
                murow = st_["rows"][0:1, :]
                srow = st_["rows"][32:33, :]
                pgux = st_.get("pgux")
                if pgux is None:
                    pgux = ps_gux.tile([D, T], f32, tag="gux")
                    pgux_passes(pgux, xnT_hi, xnT_lo, None, PGUX_TERMS,
                                start=True)
                else:
                    # sub-tiles 0/1 were accumulated inside the M1 wave
                    for gss in range(2, NSUB):
                        gsl = slice(gss * 128, (gss + 1) * 128)
                        pgux_passes(pgux, xnT_hi, xnT_lo, gsl, 2,
                                    start=True)
                pgv = ps_gv.tile([D, T], f32, tag="gv")
                for c in range(FCH):
                    nc.tensor.matmul(pgv, w2gv_sb[:, c, :], mid[:, c, :],
                                     start=(c == 0), stop=False)
                # gu.x = s*(gu.xn) + mu*sum(gu): mu rank-1 joins pgv's psum
                nc.tensor.matmul(pgv, gusum_sb, murow, start=False, stop=True)
                # s/32 broadcast to 3 partitions (ones3 = 1/32)
                s3_ps = ps_tr.tile([D, T], f32, tag="tr")
                nc.tensor.matmul(s3_ps, ones3_sb, srow, start=True, stop=True)
                s3b = gsm.tile([D, T], bf16, tag="s3")
                nc.scalar.activation(out=s3b, in_=s3_ps, func=AF.Copy)
                z_sb = gsm.tile([D, T], f32, tag="z")
                nc.vector.tensor_tensor(out=z_sb, in0=pgux, in1=s3b,
                                        op=ALU.mult)
                nc.vector.tensor_add(z_sb, z_sb, pgv)
                g_t = gsm.tile([D, T], bf16, tag="g")
                nc.scalar.activation(out=g_t, in_=z_sb, func=AF.Sigmoid,
                                     bias=gb3_sb)
                st_["g_t"] = g_t

            def stage_c2(mt, st_):
                """Gate/16 broadcast + gmid fp8 (emitted after the next
                tile's M1 so the sigmoid-chain latency never blocks PE)."""
                mid, g_t = st_["mid"], st_["g_t"]
                gb128 = gbp.tile([128, D, T], bf16, tag="gb")
                for d in range(D):
                    p_b = ps_tr.tile([128, T], f32, tag="tr")
                    nc.tensor.matmul(p_b, oh_sb[:, d * 128:(d + 1) * 128],
                                     g_t, start=True, stop=True)
                    nc.scalar.activation(out=gb128[:, d, :], in_=p_b,
                                         func=AF.Copy)
                gmid = gmp.tile([128, FCH, T], f8, tag="gmid")
                for c in range(FCH):
                    nc.gpsimd.tensor_tensor(out=gmid[:, c, :],
                                            in0=mid[:, c, :],
                                            in1=gb128[:, c // 2, :],
                                            op=ALU.mult)
                st_["gmid"] = gmid

            def stage_d(mt, st_):
                """M2 accumulates all domains (+gate*b2) + final out.
                Outputs go out as ss-pairs on the idle sync queue (HWDGE
                descriptor gen on SP, not the Pool engine)."""
                gmid, x_t = st_["gmid"], st_["x_t"]
                for ss in range(NSUB):
                    if ss % 2 == 0:
                        out_sb = outp.tile([128, 2, H], f32, tag="osb")
                    tsl = slice(ss * 128, (ss + 1) * 128)
                    for nch in range(NCH):
                        hsl = slice(nch * 512, (nch + 1) * 512)
                        po = ps_m2.tile([128, 512], f32, tag="m2")
                        i = 0
                        for w_sb in (w2hi_sb, w2lo_sb):
                            for c3 in range(FP):
                                i += 1
                                nc.tensor.matmul(
                                    po, gmid[:, 2 * c3:2 * c3 + 2, tsl],
                                    w_sb[:, 2 * c3:2 * c3 + 2, hsl],
                                    start=(i == 1),
                                    stop=(i == 2 * FP and not has_b2),
                                    perf_mode=DR)
                        if has_b2:
                            nc.tensor.matmul(po, st_["g_t"][:, tsl],
                                             b2r_sb[:, hsl],
                                             start=False, stop=True)
                        # out = 2*x + pout (reads PSUM -> DVE only)
                        nc.vector.scalar_tensor_tensor(
                            out=out_sb[:, ss % 2, hsl],
                            in0=x_t[:, ss, hsl],
                            scalar=2.0, in1=po, op0=ALU.mult, op1=ALU.add)
                    if ss % 2 == 1:
                        nc.sync.dma_start(
                            out=out_mt[mt][:, ss - 1:ss + 1, :],
                            in_=out_sb)

            # software-pipelined emission
            S = [None] * NMT
            X = [None] * NMT
            X[0] = stage_load(0, x_pre=x_first)
            S[0] = stage_a(0, X[0])
            stage_b(0, S[0], sliced=True)
            X[1] = stage_load(1, prev_x=X[0])
            # w2hi needed from stage_d(0): gate behind mt1's x arrival
            nc.gpsimd.tensor_copy(w2hi_sb[0:1, 0, 0:1], X[1][0:1, 0, 0:1])
            nc.sync.dma_start(out=w2hi_sb.rearrange("p a b -> p (a b)"),
                              in_=w2hi.ap().rearrange("p a b -> p (a b)"))
            S[1] = stage_a(1, X[1])
            stage_c(0, S[0])
            stage_b(1, S[1])
            stage_c2(0, S[0])
            X[2] = stage_load(2, prev_x=X[1])
            nc.gpsimd.tensor_copy(w2lo_sb[0:1, 0, 0:1], X[2][0:1, 0, 0:1])
            nc.sync.dma_start(out=w2lo_sb.rearrange("p a b -> p (a b)"),
                              in_=w2lo.ap().rearrange("p a b -> p (a b)"))
            S[2] = stage_a(2, X[2])
            stage_d(0, S[0])
            stage_c(1, S[1])
            stage_b(2, S[2])
            stage_c2(1, S[1])
            X[3] = stage_load(3, prev_x=X[2])
            S[3] = stage_a(3, X[3])
            stage_d(1, S[1])
            stage_c(2, S[2])
            stage_b(3, S[3])
            stage_c2(2, S[2])
            stage_d(2, S[2])
            stage_c(3, S[3])
            stage_c2(3, S[3])
            stage_d(3, S[3])

    _split_multiwaits(nc)
    return nc


last_results = None

_built = {}


def _get_nc(has_b1e, has_b2):
    key = (has_b1e, has_b2)
    if key not in _built:
        _built[key] = _build(*key)
    return _built[key]


def _to_bf16(a):
    from ml_dtypes import bfloat16
    return np.asarray(a, dtype=np.float32).astype(bfloat16)


def _to_f8(a):
    from ml_dtypes import float8_e4m3
    return np.asarray(a, dtype=np.float32).astype(float8_e4m3)


def _split_f8(a):
    from ml_dtypes import float8_e4m3
    a = np.asarray(a, dtype=np.float32)
    hi = a.astype(float8_e4m3)
    lo = (a - hi.astype(np.float32)).astype(float8_e4m3)
    return hi, lo


def kernel(x, ln_g, ln_b, W1, b1, W2, b2, gu, gv, gb):
    x = np.asarray(x, dtype=np.float32)
    ln_g = np.asarray(ln_g, dtype=np.float32)
    ln_b = np.asarray(ln_b, dtype=np.float32)
    W1 = np.asarray(W1, dtype=np.float32)
    b1 = np.asarray(b1, dtype=np.float32)
    W2 = np.asarray(W2, dtype=np.float32)
    b2 = np.asarray(b2, dtype=np.float32)
    gu = np.asarray(gu, dtype=np.float32)
    gv = np.asarray(gv, dtype=np.float32)
    gb = np.asarray(gb, dtype=np.float32)

    # ---- host precompute (all small: ~D*F*H) ----
    W1G = W1 * ln_g[:, None, :]                                # [D, F, H]
    b1e = b1 + np.einsum('dfh,dh->df', W1, ln_b)               # [D, F]
    w2gv = np.einsum('dh,dhf->df', gv, W2)                     # [D, F]
    gusum = gu.sum(axis=1)                                     # [D]
    gb_eff = gb + np.einsum('dh,dh->d', gv, b2)                # [D]

    has_b1e = bool(np.any(b1e != 0.0))
    has_b2 = bool(np.any(b2 != 0.0))

    # lhsT for M1: [128, KCH, DF]; col c*128+j = W1s[d(c), fh(c)*128+j, h]
    w1s = np.zeros((128, KCH, DF), dtype=np.float32)
    W1S = W1SC * W1G
    for c in range(FCH):
        d, fh = c // 2, c % 2
        w1s[:, :, c * 128:(c + 1) * 128] = (
            W1S[d].T.reshape(KCH, 128, F)[:, :, fh * 128:(fh + 1) * 128]
            .transpose(1, 0, 2))
    w1hi_in, w1lo_in = _split_f8(w1s)
    w1hi_in = w1hi_in.reshape(128, KP, 2, DF)
    w1lo_in = w1lo_in.reshape(128, KP, 2, DF)

    # W2 rhs for M2: [128, FCH, H]; w2t[p, c, h] = W2s[d, h, fh*128+p]
    w2s = np.zeros((128, FCH, H), dtype=np.float32)
    W2S = W2SC * W2
    for c in range(FCH):
        d, fh = c // 2, c % 2
        w2s[:, c, :] = W2S[d, :, fh * 128:(fh + 1) * 128].T
    w2hi_in, w2lo_in = _split_f8(w2s)

    # gu pack for pgux: [128, KCH, D] scaled by W1SC, fp8 hi/lo
    gus = np.ascontiguousarray(
        (W1SC * gu).T.reshape(KCH, 128, D).transpose(1, 0, 2))
    gus_hi, gus_lo = _split_f8(gus)
    gq_in = np.zeros((128, KCH, 32), dtype=gus_hi.dtype)
    gq_in[:, :, 0:D] = gus_hi
    gq_in[:, :, 16:16 + D] = gus_lo

    cpb_in = np.zeros((128, 560), dtype=np.float32)
    for d in range(D):
        cpb_in[d, d * 128:(d + 1) * 128] = 1.0 / W2SC          # gate/16 bcast
    w2gv_in = np.zeros((128, FCH, D), dtype=np.float32)
    for c in range(FCH):
        d, fh = c // 2, c % 2
        w2gv_in[:, c, d] = w2gv[d, fh * 128:(fh + 1) * 128]
    cpb_in[:, 408:426] = w2gv_in.reshape(128, FCH * D)
    cpb_in[0, 426:429] = gusum
    cpb_in[32, 429:432] = 1.0 / W1SC                           # s/32 bcast
    cpb_in[:, 432:560] = np.eye(128, dtype=np.float32)         # transpose id

    cpf_in = np.zeros((128, 8), dtype=np.float32)
    if has_b1e:
        for c in range(FCH):
            d, fh = c // 2, c % 2
            cpf_in[:, c] = b1e[d, fh * 128:(fh + 1) * 128]
    cpf_in[0:D, 6] = gb_eff
    cpf_in[:, 7] = 2.0 * EPS

    nc = _get_nc(has_b1e, has_b2)

    common = {
        "w1hi": w1hi_in,
        "w1lo": w1lo_in,
        "w2hi": w2hi_in,
        "w2lo": w2lo_in,
        "gq": gq_in,
        "cpb": _to_bf16(cpb_in),
        "cpf": cpf_in,
    }
    if has_b2:
        common["b2r"] = _to_bf16(b2)
    in_maps = [dict(common, xin=np.ascontiguousarray(x[c]))
               for c in range(B)]
    res = run_bass_kernel_spmd(nc, in_maps, core_ids=list(range(B)))
    global last_results
    last_results = res
    return np.stack([res.results[c]["out"] for c in range(B)])


# revision 22
# speedup vs baseline: 1.0451x; 1.0451x over previous
"""Trainium2 Bass kernel for nn_MixtureOfAdapter (moe_routing).

Math (per token, H=1024, F=256, D=3 domains):
    mu, sd (ddof=1) over H;  s = sd + eps;  xn = (x - mu)/s
    h_d   = xn*g_d + b_d
    mid_d = relu(W1_d h_d + b1_d);  a_d = W2_d mid_d + b2_d
    gate_d = sigmoid(gu_d.x + gv_d.a_d + gb_d)
    out = 2x + sum_d gate_d * a_d

Kernel strategy (8 cores, data-parallel over batch B=8):
  - Both big GEMMs run as fp8e4 DoubleRow matmuls (0.5 cyc per moving
    row, 4x bf16 MACs/cycle; contraction = 128 partitions x 2 pair
    lanes).  Precision is held by hi+lo fp8 splitting: a value v is
    stored as v_hi = fp8(v), v_lo = fp8(v - v_hi), and the product
    (W_hi + W_lo)(x_hi + x_lo) is computed dropping only the lo*lo
    term.  The single remaining un-split tensor is gmid = fp8(mid *
    gate/16) (one ~3.6%-rms source -> ~1.3e-2 max rel err, tolerance
    2e-2; validated against the reference in numpy).
  - Weights are pre-scaled into fp8's normal range host-side:
    W1s = 32*W1*ln_g (psum descaled by Act relu scale=1/32) and
    W2s = 16*W2 (descale folded into the gate broadcast one-hot =
    1/16, so gmid = mid*gate/16 with no extra op).
  - M1: per df-chunk, 12 DR passes: W1hi@xn_hi, W1hi@xn_lo, W1lo@xn_hi
    (k-pair pairs).  Macro-tile 0 drops the W1lo term (8 passes) so the
    PE never stalls waiting for the w1lo DMA at startup; the slightly
    larger error on tokens 0..511 stays under the global bound.
  - xn is normalized to bf16, PE-transposed (1 cyc/row), then split
    psum -> (xnT_hi fp8 via Act copy, xnT_lo fp8 via DVE/Pool
    tensor_tensor subtract).
  - Gates: pgux via fp8 DR passes on (gus_hi, gus_lo); pgv = w2gv.mid
    in bf16; mu/s rank-1 corrections from a transposed (mu, s) pack;
    gate = sigmoid(pgux*(s/32) + pgv + gb_eff).
  - M2: per (sub-tile, h-chunk) 6 DR passes (3 f-pairs x {W2hi, W2lo})
    accumulate all domains; out = 2x + psum via DVE/Pool
    scalar_tensor_tensor.
  - Engine balance: the fp8 split/gmid elementwise work is spread
    across DVE, Act and Pool (gpsimd) so no engine exceeds the PE's
    ~15.8us/macro-tile.
  - DMA discipline (the TimelineSim serializes transfers on one
    DMA_ENGINES resource and charges ~630ns HWDGE descriptor gen per
    DMA): few large DMAs; startup loads x sub-tile 0 and the w1hi
    k-pair chunks first; w1lo follows; w2hi/w2lo are gated behind
    later x arrivals; steady x loads ride the SWDGE (Pool) queue paced
    by 1-element gating copies; outputs on SWDGE except the last
    macro-tile's, which go as halves on the idle sync queue.
  - Software-pipelined emission keeps each macro-tile's gate chain
    hidden behind the next tile's M1 in the PE FIFO; macro-tile 0's M1
    is emitted in 128-token column slices so the PE starts as soon as
    the first sub-tile's transpose lands.
  - _split_multiwaits rewrites >1-wait instructions (walrus limit)
    to park extra waits on Memset/Copy carrier ops.
"""

import numpy as np

import concourse.bass as bass
import concourse.mybir as mybir
import concourse.tile as tile
from concourse.bass_utils import run_bass_kernel_spmd

B, L, H, F, D = 8, 2048, 1024, 256, 3
EPS = 1e-6
T = 512                 # tokens per macro-tile
NSUB = T // 128         # 4 sub-tiles of 128 tokens
NMT = L // T            # 4 macro-tiles per core
KCH = H // 128          # 8 k-chunks over H
KP = KCH // 2           # 4 k-pair DR passes over H
FCH = (D * F) // 128    # 6 chunks over stacked (domain, F)
FP = FCH // 2           # 3 f-pair DR passes (one per domain)
NCH = H // 512          # 2 output column chunks
DF = D * F

W1SC = 32.0             # host pre-scale on W1*ln_g (and gu)
W2SC = 16.0             # host pre-scale on W2

f32 = mybir.dt.float32
bf16 = mybir.dt.bfloat16
f8 = mybir.dt.float8e4
AF = mybir.ActivationFunctionType
ALU = mybir.AluOpType
DR = mybir.MatmulPerfMode.DoubleRow

M1_TERMS = 3            # 2 = drop W1lo@xn_hi (faster, slightly less exact)
PGUX_TERMS = 3


def _split_multiwaits(nc):
    """This walrus build allows 1 sync-wait per instruction (2 for
    EventSemaphore); Tile can attach more.  Move extras onto preceding
    same-engine carrier instructions.  A bare NoOp holds the sequencer
    while it waits (stalling dispatch of everything behind it), so where
    possible the carrier is a 1-element Memset to a dead scratch column:
    a real engine instruction parks its wait in the engine wait queue
    and lets the sequencer keep dispatching."""
    import copy
    tmpl = {}
    for f in nc.m.functions:
        for bb in f.blocks:
            for inst in bb.instructions:
                if (isinstance(inst, mybir.InstMemset)
                        and inst.engine not in tmpl):
                    tmpl[inst.engine] = inst
                elif (isinstance(inst, mybir.InstActivation)
                        and inst.func == AF.Copy
                        and inst.engine not in tmpl):
                    tmpl[inst.engine] = inst

    def carrier(inst, w, j):
        t = tmpl.get(inst.engine)
        if t is not None:
            c = copy.deepcopy(t)
            c.name = f"{inst.name}-wsplit{j}"
            c.sync_info = mybir.SyncInfo(on_wait=[w], on_update=[])
            return c
        return mybir.InstNoOp(
            name=f"{inst.name}-wsplit{j}",
            engine=inst.engine,
            sync_info=mybir.SyncInfo(on_wait=[w], on_update=[]),
            ins=[], outs=[],
        )

    for f in nc.m.functions:
        for bb in f.blocks:
            new = []
            changed = False
            for inst in bb.instructions:
                si = inst.sync_info
                cap = 2 if isinstance(inst, mybir.InstEventSemaphore) else 1
                if si is not None and len(si.on_wait) > cap:
                    waits = list(si.on_wait)
                    extra, kept = waits[:-cap], waits[-cap:]
                    for j, w in enumerate(extra):
                        new.append(carrier(inst, w, j))
                    inst.sync_info = mybir.SyncInfo(
                        on_wait=kept, on_update=list(si.on_update))
                    changed = True
                new.append(inst)
            if changed:
                bb.instructions = new


def _build(has_b1e: bool, has_b2: bool):
    nc = bass.Bass(target_bir_lowering=False)

    xin = nc.dram_tensor("xin", [L, H], f32, kind="ExternalInput")
    w1hi = nc.dram_tensor("w1hi", [128, KP, 2, DF], f8, kind="ExternalInput")
    w1lo = nc.dram_tensor("w1lo", [128, KP, 2, DF], f8, kind="ExternalInput")
    w2hi = nc.dram_tensor("w2hi", [128, FCH, H], f8, kind="ExternalInput")
    w2lo = nc.dram_tensor("w2lo", [128, FCH, H], f8, kind="ExternalInput")
    gq = nc.dram_tensor("gq", [128, KCH, 32], f8, kind="ExternalInput")
    cpb = nc.dram_tensor("cpb", [128, 560], bf16, kind="ExternalInput")
    cpf = nc.dram_tensor("cpf", [128, 8], f32, kind="ExternalInput")
    if has_b2:
        b2r = nc.dram_tensor("b2r", [D, H], bf16, kind="ExternalInput")
    out = nc.dram_tensor("out", [L, H], f32, kind="ExternalOutput")

    # [L, H] viewed as [128p, sub, H] per macro-tile
    x_mt = xin.ap().rearrange("(m s p) h -> m p s h", p=128, s=NSUB)
    out_mt = out.ap().rearrange("(m s p) h -> m p s h", p=128, s=NSUB)

    with tile.TileContext(nc) as tc:
        with (
            tc.tile_pool(name="const", bufs=1) as const,
            tc.tile_pool(name="xp", bufs=3) as xp,
            tc.tile_pool(name="xnp", bufs=2) as xnp,
            tc.tile_pool(name="xtp", bufs=3) as xtp,
            tc.tile_pool(name="midp", bufs=3) as midp,
            tc.tile_pool(name="gmp", bufs=3) as gmp,
            tc.tile_pool(name="gbp", bufs=3) as gbp,
            tc.tile_pool(name="outp", bufs=4) as outp,
            tc.tile_pool(name="smalls", bufs=6) as smalls,
            tc.tile_pool(name="gsm", bufs=3) as gsm,
            tc.tile_pool(name="ps_m1", bufs=2, space="PSUM") as ps_m1,
            tc.tile_pool(name="ps_m2", bufs=2, space="PSUM") as ps_m2,
            tc.tile_pool(name="ps_gux", bufs=1, space="PSUM") as ps_gux,
            tc.tile_pool(name="ps_gv", bufs=1, space="PSUM") as ps_gv,
            tc.tile_pool(name="ps_tr", bufs=2, space="PSUM") as ps_tr,
        ):
            # scratch columns for multiwait carrier ops (dead stores)
            scratch = const.tile([128, 4], f32)
            nc.vector.memset(scratch[:, 0:1], 0.0)
            nc.gpsimd.memset(scratch[:, 2:3], 0.0)
            nc.scalar.copy(scratch[0:1, 1:2], scratch[0:1, 3:4])

            # constants on scalar queue (small, needed early)
            cpb_sb = const.tile([128, 560], bf16)
            cpf_sb = const.tile([128, 8], f32)
            gq_sb = const.tile([128, KCH, 32], f8)
            nc.scalar.dma_start(out=cpb_sb, in_=cpb.ap())
            nc.scalar.dma_start(out=cpf_sb, in_=cpf.ap())
            nc.scalar.dma_start(out=gq_sb, in_=gq.ap())
            oh_sb = cpb_sb[0:D, 0:384]              # one-hot rows = 1/16
            w2gv_sb = cpb_sb[:, 408:426].rearrange("p (c d) -> p c d", d=D)
            gusum_sb = cpb_sb[0:1, 426:429]
            ones3_sb = cpb_sb[32:33, 429:432]       # = 1/32
            ident_b = cpb_sb[:, 432:560]
            gb3_sb = cpf_sb[0:D, 6:7]
            # dual-fp8 ldweights needs pair stride >= 32: hi at cols
            # 0:3, lo at 16:19 of a 32-wide pack
            gqhi = gq_sb[:, :, 0:D]
            gqlo = gq_sb[:, :, 16:16 + D]

            # startup loads: x sub-tiles interleaved with w1hi halves so
            # the PE can start on sub-tile 0 as soon as possible
            x_first = xp.tile([128, NSUB, H], f32, tag="x")
            w1hi_sb = const.tile([128, KP, 2, DF], f8)
            w1lo_sb = const.tile([128, KP, 2, DF], f8)
            w2hi_sb = const.tile([128, FCH, H], f8)
            w2lo_sb = const.tile([128, FCH, H], f8)
            # x sub 0 in halves so bn_stats starts on the first half
            nc.sync.dma_start(out=x_first[:, 0, 0:512],
                              in_=x_mt[0][:, 0, 0:512])
            nc.sync.dma_start(out=x_first[:, 0, 512:1024],
                              in_=x_mt[0][:, 0, 512:1024])
            nc.sync.dma_start(
                out=w1hi_sb[:, 0:2].rearrange("p a b c -> p (a b c)"),
                in_=w1hi.ap()[:, 0:2].rearrange("p a b c -> p (a b c)"))
            nc.sync.dma_start(out=x_first[:, 1, :], in_=x_mt[0][:, 1, :])
            nc.sync.dma_start(
                out=w1hi_sb[:, 2:4].rearrange("p a b c -> p (a b c)"),
                in_=w1hi.ap()[:, 2:4].rearrange("p a b c -> p (a b c)"))
            nc.sync.dma_start(out=x_first[:, 2, :], in_=x_mt[0][:, 2, :])
            nc.sync.dma_start(out=x_first[:, 3, :], in_=x_mt[0][:, 3, :])
            if has_b2:
                b2r_sb = const.tile([D, H], bf16)
                nc.scalar.dma_start(out=b2r_sb, in_=b2r.ap())

            def stage_load(mt, x_pre=None, prev_x=None):
                """x load as four per-sub-tile DMAs (the per-ss compute
                chains start on first arrival); the first is paced behind
                the previous macro-tile's ss2 arrival by a gating copy."""
                if x_pre is not None:
                    x_t = x_pre
                else:
                    x_t = xp.tile([128, NSUB, H], f32, tag="x")
                    if prev_x is not None:
                        nc.gpsimd.tensor_copy(x_t[0:1, 0, 0:1],
                                              prev_x[0:1, 2, 0:1])
                    for ss in range(NSUB):
                        nc.gpsimd.dma_start(out=x_t[:, ss, :],
                                            in_=x_mt[mt][:, ss, :])
                return x_t

            def stage_a(mt, x_t):
                """stats, normalize (bf16), PE transposes, fp8 hi/lo split."""
                xn_b = xnp.tile([128, NSUB, H], bf16, tag="xn")
                xnT_hi = xtp.tile([128, KCH, T], f8, tag="xnTh")
                xnT_lo = xtp.tile([128, KCH, T], f8, tag="xnTl")
                rows_b = xtp.tile([33, T], bf16, tag="rows")
                for ss in range(NSUB):
                    xs = x_t[:, ss, :]
                    tsl = slice(ss * 128, (ss + 1) * 128)
                    st = smalls.tile([128, 2, 6], f32, tag="bnst")
                    nc.vector.bn_stats(out=st[:, 0, :], in_=xs[:, 0:512])
                    nc.vector.bn_stats(out=st[:, 1, :], in_=xs[:, 512:1024])
                    mv = smalls.tile([128, 2], f32, tag="mv")
                    nc.vector.bn_aggr(out=mv, in_=st)
                    # sc: 0=r=1/s, 1=-mu*r, 2=s.
                    # s = sqrt(var*H/(H-1) + 2e-6) ~= sd + 1e-6 to ~5e-8.
                    sc = smalls.tile([128, 4], f32, tag="sc")
                    nc.scalar.activation(out=sc[:, 2:3], in_=mv[:, 1:2],
                                         func=AF.Sqrt,
                                         scale=float(H) / (H - 1),
                                         bias=cpf_sb[:, 7:8])
                    nc.vector.reciprocal(sc[:, 0:1], sc[:, 2:3])
                    nc.vector.tensor_scalar(out=sc[:, 1:2], in0=mv[:, 0:1],
                                            scalar1=sc[:, 0:1], scalar2=-1.0,
                                            op0=ALU.mult, op1=ALU.mult)
                    # (mu, s) pack for the row transpose
                    pk = smalls.tile([128, 33], bf16, tag="pk")
                    nc.vector.tensor_copy(pk[:, 0:1], mv[:, 0:1])
                    nc.vector.tensor_copy(pk[:, 32:33], sc[:, 2:3])
                    # xn = x*(1/s) + (-mu/s), bf16 out; alternate engines
                    if ss % 2 == 0:
                        nc.scalar.activation(out=xn_b[:, ss, :], in_=xs,
                                             func=AF.Identity,
                                             scale=sc[:, 0:1],
                                             bias=sc[:, 1:2])
                    else:
                        nc.gpsimd.tensor_scalar(out=xn_b[:, ss, :], in0=xs,
                                                scalar1=sc[:, 0:1],
                                                scalar2=sc[:, 1:2],
                                                op0=ALU.mult, op1=ALU.add)
                    # PE transposes (bf16: 1 cyc/row) into one full-bank
                    # [128, 1024] psum batch, then one wide fp8 hi copy
                    # (Act) and one wide lo subtract (DVE; gpsimd cannot
                    # read PSUM)
                    ptr = ps_tr.tile([128, 1024], bf16, tag="tr")
                    for k in range(KCH):
                        nc.tensor.transpose(
                            ptr[:, k * 128:(k + 1) * 128],
                            xn_b[:, ss, k * 128:(k + 1) * 128], ident_b)
                    dst_hi = xnT_hi[:, :, tsl]
                    dst_lo = xnT_lo[:, :, tsl]
                    src = ptr.rearrange("p (q t) -> p q t", q=KCH)
                    nc.scalar.activation(out=dst_hi, in_=src, func=AF.Copy)
                    nc.vector.tensor_tensor(out=dst_lo, in0=src, in1=dst_hi,
                                            op=ALU.subtract)
                    ptr2 = ps_tr.tile([33, 128], bf16, tag="tr")
                    nc.tensor.transpose(ptr2, pk, ident_b)
                    nc.scalar.activation(out=rows_b[:, tsl], in_=ptr2,
                                         func=AF.Copy)
                return dict(x_t=x_t, xnT_hi=xnT_hi, xnT_lo=xnT_lo,
                            rows=rows_b)

            def m1_passes(p1, c, xnT_hi, xnT_lo, tsl, terms):
                """Emit the DR passes for one df-chunk into psum p1."""
                cs = slice(c * 128, (c + 1) * 128)
                first = True
                seqs = [(w1hi_sb, xnT_hi), (w1hi_sb, xnT_lo)]
                if terms >= 3:
                    seqs.append((w1lo_sb, xnT_hi))
                n = len(seqs) * KP
                i = 0
                for w_sb, x_sb in seqs:
                    for kp in range(KP):
                        i += 1
                        nc.tensor.matmul(
                            p1[:, tsl] if tsl else p1,
                            w_sb[:, kp, :, cs],
                            x_sb[:, 2 * kp:2 * kp + 2, tsl]
                            if tsl else x_sb[:, 2 * kp:2 * kp + 2, :],
                            start=first, stop=(i == n), perf_mode=DR)
                        first = False

            def pgux_passes(pgux, xnT_hi, xnT_lo, tsl, terms, start):
                seqs = [(gqhi, xnT_hi), (gqhi, xnT_lo)]
                if terms >= 3:
                    seqs.append((gqlo, xnT_hi))
                n = len(seqs) * KP
                i = 0
                first = start
                for g_sb, x_sb in seqs:
                    for kp in range(KP):
                        i += 1
                        nc.tensor.matmul(
                            pgux[:, tsl] if tsl else pgux,
                            g_sb[:, 2 * kp:2 * kp + 2, :],
                            x_sb[:, 2 * kp:2 * kp + 2, tsl]
                            if tsl else x_sb[:, 2 * kp:2 * kp + 2, :],
                            start=first, stop=(i == n), perf_mode=DR)
                        first = False

            def stage_b(mt, st_, sliced=False):
                """M1: mid = relu((W1s @ xn)/32 (+ b1e)).  Sliced mode
                (macro-tile 0) runs token slices through four psums with
                pgux appended so all work for the first two sub-tiles is
                in the PE FIFO before anything waiting on later arrivals.
                mt0 also drops the W1lo term so the PE never waits on the
                w1lo DMA."""
                xnT_hi, xnT_lo = st_["xnT_hi"], st_["xnT_lo"]
                mid = midp.tile([128, FCH, T], bf16, tag="mid")
                terms = 2 if sliced else M1_TERMS
                if sliced:
                    p1s = []
                    for ci in range(2):
                        pw = ps_m1.tile([128, T], f32, tag="m1",
                                        name=f"m1w{ci}")
                        p1s.append(pw)
                    for ci in range(2):
                        pw = ps_m2.tile([128, 512], f32, tag="m2",
                                        name=f"m2w{ci}")
                        p1s.append(pw)
                    pgux = ps_gux.tile([D, T], f32, tag="gux")
                    st_["pgux"] = pgux
                    for ss in range(NSUB):
                        tsl = slice(ss * 128, (ss + 1) * 128)
                        for c in range(4):
                            m1_passes(p1s[c], c, xnT_hi, xnT_lo, tsl, terms)
                        if ss == 1:
                            for gss in range(2):
                                gsl = slice(gss * 128, (gss + 1) * 128)
                                pgux_passes(pgux, xnT_hi, xnT_lo, gsl,
                                            PGUX_TERMS, start=True)
                    for c in range(4):
                        bias = cpf_sb[:, c:c + 1] if has_b1e else 0.0
                        nc.scalar.activation(out=mid[:, c, :], in_=p1s[c],
                                             func=AF.Relu, bias=bias,
                                             scale=1.0 / W1SC)
                    for c in range(4, FCH):
                        p1 = ps_m1.tile([128, T], f32, tag="m1")
                        for ss in range(NSUB):
                            tsl = slice(ss * 128, (ss + 1) * 128)
                            m1_passes(p1, c, xnT_hi, xnT_lo, tsl, terms)
                        bias = cpf_sb[:, c:c + 1] if has_b1e else 0.0
                        nc.scalar.activation(out=mid[:, c, :], in_=p1,
                                             func=AF.Relu, bias=bias,
                                             scale=1.0 / W1SC)
                    st_["mid"] = mid
                    return
                # steady state: chunk-major, each chunk in two half-tile
                # token groups so chunk 0 starts as soon as sub-tiles 0/1
                # are transposed+split (instead of all four)
                for c in range(FCH):
                    p1 = ps_m1.tile([128, T], f32, tag="m1")
                    m1_passes(p1, c, xnT_hi, xnT_lo, slice(0, 256), terms)
                    m1_passes(p1, c, xnT_hi, xnT_lo, slice(256, 512), terms)
                    bias = cpf_sb[:, c:c + 1] if has_b1e else 0.0
                    nc.scalar.activation(out=mid[:, c, :], in_=p1,
                                         func=AF.Relu, bias=bias,
                                         scale=1.0 / W1SC)
                st_["mid"] = mid

            def stage_c(mt, st_):
                """Gates + g_t."""
                xnT_hi, xnT_lo, mid = (st_["xnT_hi"], st_["xnT_lo"],
                                       st_["mid"])
                murow = st_["rows"][0:1, :]
                srow = st_["rows"][32:33, :]
                pgux = st_.get("pgux")
                if pgux is None:
                    pgux = ps_gux.tile([D, T], f32, tag="gux")
                    pgux_passes(pgux, xnT_hi, xnT_lo, None, PGUX_TERMS,
                                start=True)
                else:
                    # sub-tiles 0/1 were accumulated inside the M1 wave
                    for gss in range(2, NSUB):
                        gsl = slice(gss * 128, (gss + 1) * 128)
                        pgux_passes(pgux, xnT_hi, xnT_lo, gsl, PGUX_TERMS,
                                    start=True)
                pgv = ps_gv.tile([D, T], f32, tag="gv")
                for c in range(FCH):
                    nc.tensor.matmul(pgv, w2gv_sb[:, c, :], mid[:, c, :],
                                     start=(c == 0), stop=False)
                # gu.x = s*(gu.xn) + mu*sum(gu): mu rank-1 joins pgv's psum
                nc.tensor.matmul(pgv, gusum_sb, murow, start=False, stop=True)
                # s/32 broadcast to 3 partitions (ones3 = 1/32)
                s3_ps = ps_tr.tile([D, T], f32, tag="tr")
                nc.tensor.matmul(s3_ps, ones3_sb, srow, start=True, stop=True)
                s3b = gsm.tile([D, T], bf16, tag="s3")
                nc.scalar.activation(out=s3b, in_=s3_ps, func=AF.Copy)
                z_sb = gsm.tile([D, T], f32, tag="z")
                nc.vector.tensor_tensor(out=z_sb, in0=pgux, in1=s3b,
                                        op=ALU.mult)
                nc.vector.tensor_add(z_sb, z_sb, pgv)
                g_t = gsm.tile([D, T], bf16, tag="g")
                nc.scalar.activation(out=g_t, in_=z_sb, func=AF.Sigmoid,
                                     bias=gb3_sb)
                st_["g_t"] = g_t

            def stage_c2(mt, st_):
                """Gate/16 broadcast + gmid fp8 (emitted after the next
                tile's M1 so the sigmoid-chain latency never blocks PE)."""
                mid, g_t = st_["mid"], st_["g_t"]
                gb128 = gbp.tile([128, D, T], bf16, tag="gb")
                for d in range(D):
                    p_b = ps_tr.tile([128, T], f32, tag="tr")
                    nc.tensor.matmul(p_b, oh_sb[:, d * 128:(d + 1) * 128],
                                     g_t, start=True, stop=True)
                    nc.scalar.activation(out=gb128[:, d, :], in_=p_b,
                                         func=AF.Copy)
                gmid = gmp.tile([128, FCH, T], f8, tag="gmid")
                for c in range(FCH):
                    nc.gpsimd.tensor_tensor(out=gmid[:, c, :],
                                            in0=mid[:, c, :],
                                            in1=gb128[:, c // 2, :],
                                            op=ALU.mult)
                st_["gmid"] = gmid

            def stage_d(mt, st_):
                """M2 accumulates all domains (+gate*b2) + final out.
                Outputs go out as ss-pairs on the idle sync queue (HWDGE
                descriptor gen on SP, not the Pool engine)."""
                gmid, x_t = st_["gmid"], st_["x_t"]
                for ss in range(NSUB):
                    if ss % 2 == 0:
                        out_sb = outp.tile([128, 2, H], f32, tag="osb")
                    tsl = slice(ss * 128, (ss + 1) * 128)
                    for nch in range(NCH):
                        hsl = slice(nch * 512, (nch + 1) * 512)
                        po = ps_m2.tile([128, 512], f32, tag="m2")
                        i = 0
                        for w_sb in (w2hi_sb, w2lo_sb):
                            for c3 in range(FP):
                                i += 1
                                nc.tensor.matmul(
                                    po, gmid[:, 2 * c3:2 * c3 + 2, tsl],
                                    w_sb[:, 2 * c3:2 * c3 + 2, hsl],
                                    start=(i == 1),
                                    stop=(i == 2 * FP and not has_b2),
                                    perf_mode=DR)
                        if has_b2:
                            nc.tensor.matmul(po, st_["g_t"][:, tsl],
                                             b2r_sb[:, hsl],
                                             start=False, stop=True)
                        # out = 2*x + pout (reads PSUM -> DVE only)
                        nc.vector.scalar_tensor_tensor(
                            out=out_sb[:, ss % 2, hsl],
                            in0=x_t[:, ss, hsl],
                            scalar=2.0, in1=po, op0=ALU.mult, op1=ALU.add)
                    if ss % 2 == 1:
                        nc.sync.dma_start(
                            out=out_mt[mt][:, ss - 1:ss + 1, :],
                            in_=out_sb)

            # software-pipelined emission
            S = [None] * NMT
            X = [None] * NMT
            X[0] = stage_load(0, x_pre=x_first)
            S[0] = stage_a(0, X[0])
            stage_b(0, S[0], sliced=True)
            X[1] = stage_load(1, prev_x=X[0])
            # w1lo: first needed by mt1's M1 lo passes; paced behind
            # X1.ss0 so it cannot crowd out mt0/mt1 x transfers
            nc.gpsimd.tensor_copy(w1lo_sb[0:1, 0, 0, 0:1],
                                  X[1][0:1, 0, 0:1])
            nc.sync.dma_start(
                out=w1lo_sb.rearrange("p a b c -> p (a b c)"),
                in_=w1lo.ap().rearrange("p a b c -> p (a b c)"))
            # w2hi/w2lo: first needed by stage_d(0); paced behind X1.ss3
            nc.gpsimd.tensor_copy(w2hi_sb[0:1, 0, 0:1], X[1][0:1, 3, 0:1])
            nc.sync.dma_start(out=w2hi_sb.rearrange("p a b -> p (a b)"),
                              in_=w2hi.ap().rearrange("p a b -> p (a b)"))
            nc.gpsimd.tensor_copy(w2lo_sb[0:1, 0, 0:1], X[1][0:1, 3, 0:1])
            nc.sync.dma_start(out=w2lo_sb.rearrange("p a b -> p (a b)"),
                              in_=w2lo.ap().rearrange("p a b -> p (a b)"))
            S[1] = stage_a(1, X[1])
            stage_c(0, S[0])
            stage_b(1, S[1])
            stage_c2(0, S[0])
            X[2] = stage_load(2, prev_x=X[1])
            S[2] = stage_a(2, X[2])
            stage_d(0, S[0])
            stage_c(1, S[1])
            stage_b(2, S[2])
            stage_c2(1, S[1])
            X[3] = stage_load(3, prev_x=X[2])
            S[3] = stage_a(3, X[3])
            stage_d(1, S[1])
            stage_c(2, S[2])
            stage_b(3, S[3])
            stage_c2(2, S[2])
            stage_d(2, S[2])
            stage_c(3, S[3])
            stage_c2(3, S[3])
            stage_d(3, S[3])

    _split_multiwaits(nc)
    return nc


last_results = None

_built = {}


def _get_nc(has_b1e, has_b2):
    key = (has_b1e, has_b2)
    if key not in _built:
        _built[key] = _build(*key)
    return _built[key]


def _to_bf16(a):
    from ml_dtypes import bfloat16
    return np.asarray(a, dtype=np.float32).astype(bfloat16)


def _to_f8(a):
    from ml_dtypes import float8_e4m3
    return np.asarray(a, dtype=np.float32).astype(float8_e4m3)


def _split_f8(a):
    from ml_dtypes import float8_e4m3
    a = np.asarray(a, dtype=np.float32)
    hi = a.astype(float8_e4m3)
    lo = (a - hi.astype(np.float32)).astype(float8_e4m3)
    return hi, lo


def kernel(x, ln_g, ln_b, W1, b1, W2, b2, gu, gv, gb):
    x = np.asarray(x, dtype=np.float32)
    ln_g = np.asarray(ln_g, dtype=np.float32)
    ln_b = np.asarray(ln_b, dtype=np.float32)
    W1 = np.asarray(W1, dtype=np.float32)
    b1 = np.asarray(b1, dtype=np.float32)
    W2 = np.asarray(W2, dtype=np.float32)
    b2 = np.asarray(b2, dtype=np.float32)
    gu = np.asarray(gu, dtype=np.float32)
    gv = np.asarray(gv, dtype=np.float32)
    gb = np.asarray(gb, dtype=np.float32)

    # ---- host precompute (all small: ~D*F*H) ----
    W1G = W1 * ln_g[:, None, :]                                # [D, F, H]
    b1e = b1 + np.einsum('dfh,dh->df', W1, ln_b)               # [D, F]
    w2gv = np.einsum('dh,dhf->df', gv, W2)                     # [D, F]
    gusum = gu.sum(axis=1)                                     # [D]
    gb_eff = gb + np.einsum('dh,dh->d', gv, b2)                # [D]

    has_b1e = bool(np.any(b1e != 0.0))
    has_b2 = bool(np.any(b2 != 0.0))

    # lhsT for M1: [128, KCH, DF]; col c*128+j = W1s[d(c), fh(c)*128+j, h]
    w1s = np.zeros((128, KCH, DF), dtype=np.float32)
    W1S = W1SC * W1G
    for c in range(FCH):
        d, fh = c // 2, c % 2
        w1s[:, :, c * 128:(c + 1) * 128] = (
            W1S[d].T.reshape(KCH, 128, F)[:, :, fh * 128:(fh + 1) * 128]
            .transpose(1, 0, 2))
    w1hi_in, w1lo_in = _split_f8(w1s)
    w1hi_in = w1hi_in.reshape(128, KP, 2, DF)
    w1lo_in = w1lo_in.reshape(128, KP, 2, DF)

    # W2 rhs for M2: [128, FCH, H]; w2t[p, c, h] = W2s[d, h, fh*128+p]
    w2s = np.zeros((128, FCH, H), dtype=np.float32)
    W2S = W2SC * W2
    for c in range(FCH):
        d, fh = c // 2, c % 2
        w2s[:, c, :] = W2S[d, :, fh * 128:(fh + 1) * 128].T
    w2hi_in, w2lo_in = _split_f8(w2s)

    # gu pack for pgux: [128, KCH, D] scaled by W1SC, fp8 hi/lo
    gus = np.ascontiguousarray(
        (W1SC * gu).T.reshape(KCH, 128, D).transpose(1, 0, 2))
    gus_hi, gus_lo = _split_f8(gus)
    gq_in = np.zeros((128, KCH, 32), dtype=gus_hi.dtype)
    gq_in[:, :, 0:D] = gus_hi
    gq_in[:, :, 16:16 + D] = gus_lo

    cpb_in = np.zeros((128, 560), dtype=np.float32)
    for d in range(D):
        cpb_in[d, d * 128:(d + 1) * 128] = 1.0 / W2SC          # gate/16 bcast
    w2gv_in = np.zeros((128, FCH, D), dtype=np.float32)
    for c in range(FCH):
        d, fh = c // 2, c % 2
        w2gv_in[:, c, d] = w2gv[d, fh * 128:(fh + 1) * 128]
    cpb_in[:, 408:426] = w2gv_in.reshape(128, FCH * D)
    cpb_in[0, 426:429] = gusum
    cpb_in[32, 429:432] = 1.0 / W1SC                           # s/32 bcast
    cpb_in[:, 432:560] = np.eye(128, dtype=np.float32)         # transpose id

    cpf_in = np.zeros((128, 8), dtype=np.float32)
    if has_b1e:
        for c in range(FCH):
            d, fh = c // 2, c % 2
            cpf_in[:, c] = b1e[d, fh * 128:(fh + 1) * 128]
    cpf_in[0:D, 6] = gb_eff
    cpf_in[:, 7] = 2.0 * EPS

    nc = _get_nc(has_b1e, has_b2)

    common = {
        "w1hi": w1hi_in,
        "w1lo": w1lo_in,
        "w2hi": w2hi_in,
        "w2lo": w2lo_in,
        "gq": gq_in,
        "cpb": _to_bf16(cpb_in),
        "cpf": cpf_in,
    }
    if has_b2:
        common["b2r"] = _to_bf16(b2)
    in_maps = [dict(common, xin=np.ascontiguousarray(x[c]))
               for c in range(B)]
    res = run_bass_kernel_spmd(nc, in_maps, core_ids=list(range(B)))
    global last_results
    last_results = res
    return np.stack([res.results[c]["out"] for c in range(B)])


# revision 27
# speedup vs baseline: 1.1574x; 1.1075x over previous
"""Trainium2 Bass kernel for nn_MixtureOfAdapter (moe_routing).

Math (per token, H=1024, F=256, D=3 domains):
    mu, sd (ddof=1) over H;  s = sd + eps;  xn = (x - mu)/s
    h_d   = xn*g_d + b_d
    mid_d = relu(W1_d h_d + b1_d);  a_d = W2_d mid_d + b2_d
    gate_d = sigmoid(gu_d.x + gv_d.a_d + gb_d)
    out = 2x + sum_d gate_d * a_d

Kernel strategy (8 cores, data-parallel over batch B=8):
  - Both big GEMMs run as fp8e4 DoubleRow matmuls (0.5 cyc per moving
    row, 4x bf16 MACs/cycle; contraction = 128 partitions x 2 pair
    lanes).  Precision is held by hi+lo fp8 splitting: a value v is
    stored as v_hi = fp8(v), v_lo = fp8(v - v_hi), and the product
    (W_hi + W_lo)(x_hi + x_lo) is computed dropping only the lo*lo
    term.  The single remaining un-split tensor is gmid = fp8(mid *
    gate/16) (one ~3.6%-rms source -> ~1.3e-2 max rel err, tolerance
    2e-2; validated against the reference in numpy).
  - Weights are pre-scaled into fp8's normal range host-side:
    W1s = 32*W1*ln_g (psum descaled by Act relu scale=1/32) and
    W2s = 16*W2 (descale folded into the gate broadcast one-hot =
    1/16, so gmid = mid*gate/16 with no extra op).
  - M1: per df-chunk, 12 DR passes: W1hi@xn_hi, W1hi@xn_lo, W1lo@xn_hi
    (k-pair pairs).  Macro-tile 0 drops the W1lo term (8 passes) so the
    PE never stalls waiting for the w1lo DMA at startup; the slightly
    larger error on tokens 0..511 stays under the global bound.
  - xn is normalized to bf16, PE-transposed (1 cyc/row), then split
    psum -> (xnT_hi fp8 via Act copy, xnT_lo fp8 via DVE/Pool
    tensor_tensor subtract).
  - Gates: pgux via fp8 DR passes on (gus_hi, gus_lo); pgv = w2gv.mid
    in bf16; mu/s rank-1 corrections from a transposed (mu, s) pack;
    gate = sigmoid(pgux*(s/32) + pgv + gb_eff).
  - M2: per (sub-tile, h-chunk) 6 DR passes (3 f-pairs x {W2hi, W2lo})
    accumulate all domains; out = 2x + psum via DVE/Pool
    scalar_tensor_tensor.
  - Engine balance: the fp8 split/gmid elementwise work is spread
    across DVE, Act and Pool (gpsimd) so no engine exceeds the PE's
    ~15.8us/macro-tile.
  - DMA discipline (the TimelineSim serializes transfers on one
    DMA_ENGINES resource and charges ~630ns HWDGE descriptor gen per
    DMA): few large DMAs; startup loads x sub-tile 0 and the w1hi
    k-pair chunks first; w1lo follows; w2hi/w2lo are gated behind
    later x arrivals; steady x loads ride the SWDGE (Pool) queue paced
    by 1-element gating copies; outputs on SWDGE except the last
    macro-tile's, which go as halves on the idle sync queue.
  - Software-pipelined emission keeps each macro-tile's gate chain
    hidden behind the next tile's M1 in the PE FIFO; macro-tile 0's M1
    is emitted in 128-token column slices so the PE starts as soon as
    the first sub-tile's transpose lands.
  - _split_multiwaits rewrites >1-wait instructions (walrus limit)
    to park extra waits on Memset/Copy carrier ops.
"""

import numpy as np

import concourse.bass as bass
import concourse.mybir as mybir
import concourse.tile as tile
from concourse.bass_utils import run_bass_kernel_spmd

B, L, H, F, D = 8, 2048, 1024, 256, 3
EPS = 1e-6
T = 512                 # tokens per macro-tile
NSUB = T // 128         # 4 sub-tiles of 128 tokens
NMT = L // T            # 4 macro-tiles per core
KCH = H // 128          # 8 k-chunks over H
KP = KCH // 2           # 4 k-pair DR passes over H
FCH = (D * F) // 128    # 6 chunks over stacked (domain, F)
FP = FCH // 2           # 3 f-pair DR passes (one per domain)
NCH = H // 512          # 2 output column chunks
DF = D * F

W1SC = 32.0             # host pre-scale on W1*ln_g (and gu)
W2SC = 16.0             # host pre-scale on W2

f32 = mybir.dt.float32
bf16 = mybir.dt.bfloat16
f8 = mybir.dt.float8e4
AF = mybir.ActivationFunctionType
ALU = mybir.AluOpType
DR = mybir.MatmulPerfMode.DoubleRow

M1_TERMS = 3            # 2 = drop W1lo@xn_hi (faster, slightly less exact)
PGUX_TERMS = 3


def _split_multiwaits(nc):
    """This walrus build allows 1 sync-wait per instruction (2 for
    EventSemaphore); Tile can attach more.  Move extras onto preceding
    same-engine carrier instructions.  A bare NoOp holds the sequencer
    while it waits (stalling dispatch of everything behind it), so where
    possible the carrier is a 1-element Memset to a dead scratch column:
    a real engine instruction parks its wait in the engine wait queue
    and lets the sequencer keep dispatching."""
    import copy
    tmpl = {}
    for f in nc.m.functions:
        for bb in f.blocks:
            for inst in bb.instructions:
                if (isinstance(inst, mybir.InstMemset)
                        and inst.engine not in tmpl):
                    tmpl[inst.engine] = inst
                elif (isinstance(inst, mybir.InstActivation)
                        and inst.func == AF.Copy
                        and inst.engine not in tmpl):
                    tmpl[inst.engine] = inst

    def carrier(inst, w, j):
        t = tmpl.get(inst.engine)
        if t is not None:
            c = copy.deepcopy(t)
            c.name = f"{inst.name}-wsplit{j}"
            c.sync_info = mybir.SyncInfo(on_wait=[w], on_update=[])
            return c
        return mybir.InstNoOp(
            name=f"{inst.name}-wsplit{j}",
            engine=inst.engine,
            sync_info=mybir.SyncInfo(on_wait=[w], on_update=[]),
            ins=[], outs=[],
        )

    for f in nc.m.functions:
        for bb in f.blocks:
            new = []
            changed = False
            for inst in bb.instructions:
                si = inst.sync_info
                cap = 2 if isinstance(inst, mybir.InstEventSemaphore) else 1
                if si is not None and len(si.on_wait) > cap:
                    waits = list(si.on_wait)
                    extra, kept = waits[:-cap], waits[-cap:]
                    for j, w in enumerate(extra):
                        new.append(carrier(inst, w, j))
                    inst.sync_info = mybir.SyncInfo(
                        on_wait=kept, on_update=list(si.on_update))
                    changed = True
                new.append(inst)
            if changed:
                bb.instructions = new


def _build(has_b1e: bool, has_b2: bool):
    nc = bass.Bass(target_bir_lowering=False)

    xin = nc.dram_tensor("xin", [L, H], f32, kind="ExternalInput")
    w1hi = nc.dram_tensor("w1hi", [128, KP, 2, DF], f8, kind="ExternalInput")
    w1lo = nc.dram_tensor("w1lo", [128, KP, 2, DF], f8, kind="ExternalInput")
    w2hi = nc.dram_tensor("w2hi", [128, FCH, H], f8, kind="ExternalInput")
    w2lo = nc.dram_tensor("w2lo", [128, FCH, H], f8, kind="ExternalInput")
    gq = nc.dram_tensor("gq", [128, KCH, 32], f8, kind="ExternalInput")
    cpb = nc.dram_tensor("cpb", [128, 560], bf16, kind="ExternalInput")
    cpf = nc.dram_tensor("cpf", [128, 8], f32, kind="ExternalInput")
    if has_b2:
        b2r = nc.dram_tensor("b2r", [D, H], bf16, kind="ExternalInput")
    out = nc.dram_tensor("out", [L, H], f32, kind="ExternalOutput")

    # [L, H] viewed as [128p, sub, H] per macro-tile
    x_mt = xin.ap().rearrange("(m s p) h -> m p s h", p=128, s=NSUB)
    out_mt = out.ap().rearrange("(m s p) h -> m p s h", p=128, s=NSUB)

    with tile.TileContext(nc) as tc:
        with (
            tc.tile_pool(name="const", bufs=1) as const,
            tc.tile_pool(name="xp", bufs=3) as xp,
            tc.tile_pool(name="xnp", bufs=2) as xnp,
            tc.tile_pool(name="xtp", bufs=3) as xtp,
            tc.tile_pool(name="midp", bufs=3) as midp,
            tc.tile_pool(name="gmp", bufs=3) as gmp,
            tc.tile_pool(name="gbp", bufs=3) as gbp,
            tc.tile_pool(name="outp", bufs=4) as outp,
            tc.tile_pool(name="smalls", bufs=6) as smalls,
            tc.tile_pool(name="gsm", bufs=3) as gsm,
            tc.tile_pool(name="ps_m1", bufs=2, space="PSUM") as ps_m1,
            tc.tile_pool(name="ps_m2", bufs=2, space="PSUM") as ps_m2,
            tc.tile_pool(name="ps_gux", bufs=1, space="PSUM") as ps_gux,
            tc.tile_pool(name="ps_gv", bufs=1, space="PSUM") as ps_gv,
            tc.tile_pool(name="ps_tr", bufs=2, space="PSUM") as ps_tr,
        ):
            # scratch columns for multiwait carrier ops (dead stores)
            scratch = const.tile([128, 4], f32)
            nc.vector.memset(scratch[:, 0:1], 0.0)
            nc.gpsimd.memset(scratch[:, 2:3], 0.0)
            nc.scalar.copy(scratch[0:1, 1:2], scratch[0:1, 3:4])

            # constants on scalar queue (small, needed early)
            cpb_sb = const.tile([128, 560], bf16)
            cpf_sb = const.tile([128, 8], f32)
            gq_sb = const.tile([128, KCH, 32], f8)
            nc.scalar.dma_start(out=cpb_sb, in_=cpb.ap())
            nc.scalar.dma_start(out=cpf_sb, in_=cpf.ap())
            nc.scalar.dma_start(out=gq_sb, in_=gq.ap())
            oh_sb = cpb_sb[0:D, 0:384]              # one-hot rows = 1/16
            w2gv_sb = cpb_sb[:, 408:426].rearrange("p (c d) -> p c d", d=D)
            gusum_sb = cpb_sb[0:1, 426:429]
            ones3_sb = cpb_sb[32:33, 429:432]       # = 1/32
            ident_b = cpb_sb[:, 432:560]
            gb3_sb = cpf_sb[0:D, 6:7]
            # dual-fp8 ldweights needs pair stride >= 32: hi at cols
            # 0:3, lo at 16:19 of a 32-wide pack
            gqhi = gq_sb[:, :, 0:D]
            gqlo = gq_sb[:, :, 16:16 + D]

            # startup loads: x sub-tiles interleaved with w1hi halves so
            # the PE can start on sub-tile 0 as soon as possible
            x_first = xp.tile([128, NSUB, H], f32, tag="x")
            w1hi_sb = const.tile([128, KP, 2, DF], f8)
            w1lo_sb = const.tile([128, KP, 2, DF], f8)
            w2hi_sb = const.tile([128, FCH, H], f8)
            w2lo_sb = const.tile([128, FCH, H], f8)
            # x sub 0 in halves so bn_stats starts on the first half
            nc.sync.dma_start(out=x_first[:, 0, 0:512],
                              in_=x_mt[0][:, 0, 0:512])
            nc.sync.dma_start(out=x_first[:, 0, 512:1024],
                              in_=x_mt[0][:, 0, 512:1024])
            nc.sync.dma_start(
                out=w1hi_sb[:, 0:2].rearrange("p a b c -> p (a b c)"),
                in_=w1hi.ap()[:, 0:2].rearrange("p a b c -> p (a b c)"))
            nc.sync.dma_start(out=x_first[:, 1, :], in_=x_mt[0][:, 1, :])
            nc.sync.dma_start(
                out=w1hi_sb[:, 2:4].rearrange("p a b c -> p (a b c)"),
                in_=w1hi.ap()[:, 2:4].rearrange("p a b c -> p (a b c)"))
            nc.sync.dma_start(out=x_first[:, 2, :], in_=x_mt[0][:, 2, :])
            nc.sync.dma_start(out=x_first[:, 3, :], in_=x_mt[0][:, 3, :])
            if has_b2:
                b2r_sb = const.tile([D, H], bf16)
                nc.scalar.dma_start(out=b2r_sb, in_=b2r.ap())

            def stage_load(mt, x_pre=None, prev_x=None, tail=None):
                """x load as four per-sub-tile DMAs on the sync queue (SP
                pays the HWDGE descriptor gen; the Pool engine only runs
                the 95ns pacing gate copies).  The first pair is paced
                behind the previous macro-tile's ss2 arrival; `tail` emits
                extra gated DMAs between the ss1 and ss2 loads (used to
                slot w1lo in front of mt1's later sub-tiles)."""
                if x_pre is not None:
                    x_t = x_pre
                else:
                    x_t = xp.tile([128, NSUB, H], f32, tag="x")
                    if prev_x is not None:
                        nc.gpsimd.tensor_copy(x_t[0:1, 0, 0:1],
                                              prev_x[0:1, 2, 0:1])
                    for ss in range(2):
                        nc.sync.dma_start(out=x_t[:, ss, :],
                                          in_=x_mt[mt][:, ss, :])
                    if tail is not None:
                        tail(x_t)
                    for ss in range(2, NSUB):
                        nc.sync.dma_start(out=x_t[:, ss, :],
                                          in_=x_mt[mt][:, ss, :])
                return x_t

            def stage_a(mt, x_t):
                """stats, normalize (bf16), PE transposes, fp8 hi/lo split."""
                xn_b = xnp.tile([128, NSUB, H], bf16, tag="xn")
                xnT_hi = xtp.tile([128, KCH, T], f8, tag="xnTh")
                xnT_lo = xtp.tile([128, KCH, T], f8, tag="xnTl")
                rows_b = xtp.tile([33, T], bf16, tag="rows")
                for ss in range(NSUB):
                    xs = x_t[:, ss, :]
                    tsl = slice(ss * 128, (ss + 1) * 128)
                    st = smalls.tile([128, 2, 6], f32, tag="bnst")
                    nc.vector.bn_stats(out=st[:, 0, :], in_=xs[:, 0:512])
                    nc.vector.bn_stats(out=st[:, 1, :], in_=xs[:, 512:1024])
                    mv = smalls.tile([128, 2], f32, tag="mv")
                    nc.vector.bn_aggr(out=mv, in_=st)
                    # sc: 0=r=1/s, 1=-mu*r, 2=s.
                    # s = sqrt(var*H/(H-1) + 2e-6) ~= sd + 1e-6 to ~5e-8.
                    sc = smalls.tile([128, 4], f32, tag="sc")
                    nc.scalar.activation(out=sc[:, 2:3], in_=mv[:, 1:2],
                                         func=AF.Sqrt,
                                         scale=float(H) / (H - 1),
                                         bias=cpf_sb[:, 7:8])
                    nc.vector.reciprocal(sc[:, 0:1], sc[:, 2:3])
                    nc.vector.tensor_scalar(out=sc[:, 1:2], in0=mv[:, 0:1],
                                            scalar1=sc[:, 0:1], scalar2=-1.0,
                                            op0=ALU.mult, op1=ALU.mult)
                    # (mu, s) pack for the row transpose
                    pk = smalls.tile([128, 33], bf16, tag="pk")
                    nc.vector.tensor_copy(pk[:, 0:1], mv[:, 0:1])
                    nc.vector.tensor_copy(pk[:, 32:33], sc[:, 2:3])
                    # xn = x*(1/s) + (-mu/s), bf16 out; alternate engines
                    if ss % 2 == 0:
                        nc.scalar.activation(out=xn_b[:, ss, :], in_=xs,
                                             func=AF.Identity,
                                             scale=sc[:, 0:1],
                                             bias=sc[:, 1:2])
                    else:
                        nc.gpsimd.tensor_scalar(out=xn_b[:, ss, :], in0=xs,
                                                scalar1=sc[:, 0:1],
                                                scalar2=sc[:, 1:2],
                                                op0=ALU.mult, op1=ALU.add)
                    # PE transposes (bf16: 1 cyc/row) into one full-bank
                    # [128, 1024] psum batch, then one wide fp8 hi copy
                    # (Act) and one wide lo subtract (DVE; gpsimd cannot
                    # read PSUM)
                    ptr = ps_tr.tile([128, 1024], bf16, tag="tr")
                    for k in range(KCH):
                        nc.tensor.transpose(
                            ptr[:, k * 128:(k + 1) * 128],
                            xn_b[:, ss, k * 128:(k + 1) * 128], ident_b)
                    dst_hi = xnT_hi[:, :, tsl]
                    dst_lo = xnT_lo[:, :, tsl]
                    src = ptr.rearrange("p (q t) -> p q t", q=KCH)
                    nc.scalar.activation(out=dst_hi, in_=src, func=AF.Copy)
                    nc.vector.tensor_tensor(out=dst_lo, in0=src, in1=dst_hi,
                                            op=ALU.subtract)
                    ptr2 = ps_tr.tile([33, 128], bf16, tag="tr")
                    nc.tensor.transpose(ptr2, pk, ident_b)
                    nc.scalar.activation(out=rows_b[:, tsl], in_=ptr2,
                                         func=AF.Copy)
                return dict(x_t=x_t, xnT_hi=xnT_hi, xnT_lo=xnT_lo,
                            rows=rows_b)

            def m1_passes(p1, c, xnT_hi, xnT_lo, tsl, terms):
                """Emit the DR passes for one df-chunk into psum p1."""
                cs = slice(c * 128, (c + 1) * 128)
                first = True
                seqs = [(w1hi_sb, xnT_hi), (w1hi_sb, xnT_lo)]
                if terms >= 3:
                    seqs.append((w1lo_sb, xnT_hi))
                n = len(seqs) * KP
                i = 0
                for w_sb, x_sb in seqs:
                    for kp in range(KP):
                        i += 1
                        nc.tensor.matmul(
                            p1[:, tsl] if tsl else p1,
                            w_sb[:, kp, :, cs],
                            x_sb[:, 2 * kp:2 * kp + 2, tsl]
                            if tsl else x_sb[:, 2 * kp:2 * kp + 2, :],
                            start=first, stop=(i == n), perf_mode=DR)
                        first = False

            def pgux_passes(pgux, xnT_hi, xnT_lo, tsl, terms, start):
                seqs = [(gqhi, xnT_hi), (gqhi, xnT_lo)]
                if terms >= 3:
                    seqs.append((gqlo, xnT_hi))
                n = len(seqs) * KP
                i = 0
                first = start
                for g_sb, x_sb in seqs:
                    for kp in range(KP):
                        i += 1
                        nc.tensor.matmul(
                            pgux[:, tsl] if tsl else pgux,
                            g_sb[:, 2 * kp:2 * kp + 2, :],
                            x_sb[:, 2 * kp:2 * kp + 2, tsl]
                            if tsl else x_sb[:, 2 * kp:2 * kp + 2, :],
                            start=first, stop=(i == n), perf_mode=DR)
                        first = False

            def stage_b(mt, st_, sliced=False):
                """M1: mid = relu((W1s @ xn)/32 (+ b1e)).  Sliced mode
                (macro-tile 0) runs token slices through four psums with
                pgux appended so all work for the first two sub-tiles is
                in the PE FIFO before anything waiting on later arrivals.
                mt0 also drops the W1lo term so the PE never waits on the
                w1lo DMA."""
                xnT_hi, xnT_lo = st_["xnT_hi"], st_["xnT_lo"]
                mid = midp.tile([128, FCH, T], bf16, tag="mid")
                terms = 2 if sliced else M1_TERMS
                if sliced:
                    p1s = []
                    for ci in range(2):
                        pw = ps_m1.tile([128, T], f32, tag="m1",
                                        name=f"m1w{ci}")
                        p1s.append(pw)
                    for ci in range(2):
                        pw = ps_m2.tile([128, 512], f32, tag="m2",
                                        name=f"m2w{ci}")
                        p1s.append(pw)
                    pgux = ps_gux.tile([D, T], f32, tag="gux")
                    st_["pgux"] = pgux
                    for ss in range(NSUB):
                        tsl = slice(ss * 128, (ss + 1) * 128)
                        for c in range(4):
                            m1_passes(p1s[c], c, xnT_hi, xnT_lo, tsl, terms)
                        if ss == 1:
                            for gss in range(2):
                                gsl = slice(gss * 128, (gss + 1) * 128)
                                pgux_passes(pgux, xnT_hi, xnT_lo, gsl,
                                            PGUX_TERMS, start=True)
                    for c in range(4):
                        bias = cpf_sb[:, c:c + 1] if has_b1e else 0.0
                        nc.scalar.activation(out=mid[:, c, :], in_=p1s[c],
                                             func=AF.Relu, bias=bias,
                                             scale=1.0 / W1SC)
                    for c in range(4, FCH):
                        p1 = ps_m1.tile([128, T], f32, tag="m1")
                        for ss in range(NSUB):
                            tsl = slice(ss * 128, (ss + 1) * 128)
                            m1_passes(p1, c, xnT_hi, xnT_lo, tsl, terms)
                        bias = cpf_sb[:, c:c + 1] if has_b1e else 0.0
                        nc.scalar.activation(out=mid[:, c, :], in_=p1,
                                             func=AF.Relu, bias=bias,
                                             scale=1.0 / W1SC)
                    st_["mid"] = mid
                    return
                # steady state: chunk-major, each chunk in two half-tile
                # token groups so chunk 0 starts as soon as sub-tiles 0/1
                # are transposed+split (instead of all four)
                for c in range(FCH):
                    p1 = ps_m1.tile([128, T], f32, tag="m1")
                    m1_passes(p1, c, xnT_hi, xnT_lo, slice(0, 256), terms)
                    m1_passes(p1, c, xnT_hi, xnT_lo, slice(256, 512), terms)
                    bias = cpf_sb[:, c:c + 1] if has_b1e else 0.0
                    nc.scalar.activation(out=mid[:, c, :], in_=p1,
                                         func=AF.Relu, bias=bias,
                                         scale=1.0 / W1SC)
                st_["mid"] = mid

            def stage_c(mt, st_):
                """Gates + g_t."""
                xnT_hi, xnT_lo, mid = (st_["xnT_hi"], st_["xnT_lo"],
                                       st_["mid"])
                murow = st_["rows"][0:1, :]
                srow = st_["rows"][32:33, :]
                pgux = st_.get("pgux")
                if pgux is None:
                    pgux = ps_gux.tile([D, T], f32, tag="gux")
                    pgux_passes(pgux, xnT_hi, xnT_lo, None, PGUX_TERMS,
                                start=True)
                else:
                    # sub-tiles 0/1 were accumulated inside the M1 wave
                    for gss in range(2, NSUB):
                        gsl = slice(gss * 128, (gss + 1) * 128)
                        pgux_passes(pgux, xnT_hi, xnT_lo, gsl, PGUX_TERMS,
                                    start=True)
                pgv = ps_gv.tile([D, T], f32, tag="gv")
                for c in range(FCH):
                    nc.tensor.matmul(pgv, w2gv_sb[:, c, :], mid[:, c, :],
                                     start=(c == 0), stop=False)
                # gu.x = s*(gu.xn) + mu*sum(gu): mu rank-1 joins pgv's psum
                nc.tensor.matmul(pgv, gusum_sb, murow, start=False, stop=True)
                # s/32 broadcast to 3 partitions (ones3 = 1/32)
                s3_ps = ps_tr.tile([D, T], f32, tag="tr")
                nc.tensor.matmul(s3_ps, ones3_sb, srow, start=True, stop=True)
                s3b = gsm.tile([D, T], bf16, tag="s3")
                nc.scalar.activation(out=s3b, in_=s3_ps, func=AF.Copy)
                z_sb = gsm.tile([D, T], f32, tag="z")
                nc.vector.tensor_tensor(out=z_sb, in0=pgux, in1=s3b,
                                        op=ALU.mult)
                nc.vector.tensor_add(z_sb, z_sb, pgv)
                g_t = gsm.tile([D, T], bf16, tag="g")
                nc.scalar.activation(out=g_t, in_=z_sb, func=AF.Sigmoid,
                                     bias=gb3_sb)
                st_["g_t"] = g_t

            def stage_c2(mt, st_):
                """Gate/16 broadcast + gmid fp8 (emitted after the next
                tile's M1 so the sigmoid-chain latency never blocks PE)."""
                mid, g_t = st_["mid"], st_["g_t"]
                gb128 = gbp.tile([128, D, T], bf16, tag="gb")
                for d in range(D):
                    p_b = ps_tr.tile([128, T], f32, tag="tr")
                    nc.tensor.matmul(p_b, oh_sb[:, d * 128:(d + 1) * 128],
                                     g_t, start=True, stop=True)
                    nc.scalar.activation(out=gb128[:, d, :], in_=p_b,
                                         func=AF.Copy)
                gmid = gmp.tile([128, FCH, T], f8, tag="gmid")
                for c in range(FCH):
                    # first f-pair on DVE (shorter latency) so M2's first
                    # Ldweights isn't gated on the slower Pool queue
                    eng = nc.vector if c < 2 else nc.gpsimd
                    eng.tensor_tensor(out=gmid[:, c, :],
                                      in0=mid[:, c, :],
                                      in1=gb128[:, c // 2, :],
                                      op=ALU.mult)
                st_["gmid"] = gmid

            def stage_d(mt, st_):
                """M2 accumulates all domains (+gate*b2) + final out.
                Outputs go out as ss-pairs on the idle sync queue (HWDGE
                descriptor gen on SP, not the Pool engine)."""
                gmid, x_t = st_["gmid"], st_["x_t"]
                for ss in range(NSUB):
                    if ss % 2 == 0:
                        out_sb = outp.tile([128, 2, H], f32, tag="osb")
                    tsl = slice(ss * 128, (ss + 1) * 128)
                    for nch in range(NCH):
                        hsl = slice(nch * 512, (nch + 1) * 512)
                        po = ps_m2.tile([128, 512], f32, tag="m2")
                        i = 0
                        for w_sb in (w2hi_sb, w2lo_sb):
                            for c3 in range(FP):
                                i += 1
                                nc.tensor.matmul(
                                    po, gmid[:, 2 * c3:2 * c3 + 2, tsl],
                                    w_sb[:, 2 * c3:2 * c3 + 2, hsl],
                                    start=(i == 1),
                                    stop=(i == 2 * FP and not has_b2),
                                    perf_mode=DR)
                        if has_b2:
                            nc.tensor.matmul(po, st_["g_t"][:, tsl],
                                             b2r_sb[:, hsl],
                                             start=False, stop=True)
                        # out = 2*x + pout (reads PSUM -> DVE only)
                        nc.vector.scalar_tensor_tensor(
                            out=out_sb[:, ss % 2, hsl],
                            in0=x_t[:, ss, hsl],
                            scalar=2.0, in1=po, op0=ALU.mult, op1=ALU.add)
                    if mt == NMT - 1:
                        # last tile: store per sub-tile half-pair so the
                        # final transfer is small and starts early
                        nc.sync.dma_start(out=out_mt[mt][:, ss, :],
                                          in_=out_sb[:, ss % 2, :])
                    elif ss % 2 == 1:
                        nc.sync.dma_start(
                            out=out_mt[mt][:, ss - 1:ss + 1, :],
                            in_=out_sb)

            # software-pipelined emission
            S = [None] * NMT
            X = [None] * NMT
            X[0] = stage_load(0, x_pre=x_first)
            S[0] = stage_a(0, X[0])
            stage_b(0, S[0], sliced=True)
            def _w1lo_tail(x_t):
                # w1lo: first needed by mt1's M1 lo passes; slot its
                # transfer between mt1's ss1 and ss2 loads
                nc.gpsimd.tensor_copy(w1lo_sb[0:1, 0, 0, 0:1],
                                      x_t[0:1, 0, 0:1])
                nc.sync.dma_start(
                    out=w1lo_sb.rearrange("p a b c -> p (a b c)"),
                    in_=w1lo.ap().rearrange("p a b c -> p (a b c)"))

            X[1] = stage_load(1, prev_x=X[0], tail=_w1lo_tail)
            # w2hi/w2lo: first needed by stage_d(0); paced behind X1.ss3
            nc.gpsimd.tensor_copy(w2hi_sb[0:1, 0, 0:1], X[1][0:1, 3, 0:1])
            nc.sync.dma_start(out=w2hi_sb.rearrange("p a b -> p (a b)"),
                              in_=w2hi.ap().rearrange("p a b -> p (a b)"))
            nc.gpsimd.tensor_copy(w2lo_sb[0:1, 0, 0:1], X[1][0:1, 3, 0:1])
            nc.sync.dma_start(out=w2lo_sb.rearrange("p a b -> p (a b)"),
                              in_=w2lo.ap().rearrange("p a b -> p (a b)"))
            S[1] = stage_a(1, X[1])
            stage_c(0, S[0])
            stage_b(1, S[1])
            stage_c2(0, S[0])
            X[2] = stage_load(2, prev_x=X[1])
            S[2] = stage_a(2, X[2])
            X[3] = stage_load(3, prev_x=X[2])
            stage_d(0, S[0])
            stage_c(1, S[1])
            stage_b(2, S[2])
            stage_c2(1, S[1])
            S[3] = stage_a(3, X[3])
            stage_d(1, S[1])
            stage_c(2, S[2])
            stage_b(3, S[3])
            stage_c2(2, S[2])
            stage_d(2, S[2])
            stage_c(3, S[3])
            stage_c2(3, S[3])
            stage_d(3, S[3])

    _split_multiwaits(nc)
    return nc


last_results = None

_built = {}


def _get_nc(has_b1e, has_b2):
    key = (has_b1e, has_b2)
    if key not in _built:
        _built[key] = _build(*key)
    return _built[key]


def _to_bf16(a):
    from ml_dtypes import bfloat16
    return np.asarray(a, dtype=np.float32).astype(bfloat16)


def _to_f8(a):
    from ml_dtypes import float8_e4m3
    return np.asarray(a, dtype=np.float32).astype(float8_e4m3)


def _split_f8(a):
    from ml_dtypes import float8_e4m3
    a = np.asarray(a, dtype=np.float32)
    hi = a.astype(float8_e4m3)
    lo = (a - hi.astype(np.float32)).astype(float8_e4m3)
    return hi, lo


def kernel(x, ln_g, ln_b, W1, b1, W2, b2, gu, gv, gb):
    x = np.asarray(x, dtype=np.float32)
    ln_g = np.asarray(ln_g, dtype=np.float32)
    ln_b = np.asarray(ln_b, dtype=np.float32)
    W1 = np.asarray(W1, dtype=np.float32)
    b1 = np.asarray(b1, dtype=np.float32)
    W2 = np.asarray(W2, dtype=np.float32)
    b2 = np.asarray(b2, dtype=np.float32)
    gu = np.asarray(gu, dtype=np.float32)
    gv = np.asarray(gv, dtype=np.float32)
    gb = np.asarray(gb, dtype=np.float32)

    # ---- host precompute (all small: ~D*F*H) ----
    W1G = W1 * ln_g[:, None, :]                                # [D, F, H]
    b1e = b1 + np.einsum('dfh,dh->df', W1, ln_b)               # [D, F]
    w2gv = np.einsum('dh,dhf->df', gv, W2)                     # [D, F]
    gusum = gu.sum(axis=1)                                     # [D]
    gb_eff = gb + np.einsum('dh,dh->d', gv, b2)                # [D]

    has_b1e = bool(np.any(b1e != 0.0))
    has_b2 = bool(np.any(b2 != 0.0))

    # lhsT for M1: [128, KCH, DF]; col c*128+j = W1s[d(c), fh(c)*128+j, h]
    w1s = np.zeros((128, KCH, DF), dtype=np.float32)
    W1S = W1SC * W1G
    for c in range(FCH):
        d, fh = c // 2, c % 2
        w1s[:, :, c * 128:(c + 1) * 128] = (
            W1S[d].T.reshape(KCH, 128, F)[:, :, fh * 128:(fh + 1) * 128]
            .transpose(1, 0, 2))
    w1hi_in, w1lo_in = _split_f8(w1s)
    w1hi_in = w1hi_in.reshape(128, KP, 2, DF)
    w1lo_in = w1lo_in.reshape(128, KP, 2, DF)

    # W2 rhs for M2: [128, FCH, H]; w2t[p, c, h] = W2s[d, h, fh*128+p]
    w2s = np.zeros((128, FCH, H), dtype=np.float32)
    W2S = W2SC * W2
    for c in range(FCH):
        d, fh = c // 2, c % 2
        w2s[:, c, :] = W2S[d, :, fh * 128:(fh + 1) * 128].T
    w2hi_in, w2lo_in = _split_f8(w2s)

    # gu pack for pgux: [128, KCH, D] scaled by W1SC, fp8 hi/lo
    gus = np.ascontiguousarray(
        (W1SC * gu).T.reshape(KCH, 128, D).transpose(1, 0, 2))
    gus_hi, gus_lo = _split_f8(gus)
    gq_in = np.zeros((128, KCH, 32), dtype=gus_hi.dtype)
    gq_in[:, :, 0:D] = gus_hi
    gq_in[:, :, 16:16 + D] = gus_lo

    cpb_in = np.zeros((128, 560), dtype=np.float32)
    for d in range(D):
        cpb_in[d, d * 128:(d + 1) * 128] = 1.0 / W2SC          # gate/16 bcast
    w2gv_in = np.zeros((128, FCH, D), dtype=np.float32)
    for c in range(FCH):
        d, fh = c // 2, c % 2
        w2gv_in[:, c, d] = w2gv[d, fh * 128:(fh + 1) * 128]
    cpb_in[:, 408:426] = w2gv_in.reshape(128, FCH * D)
    cpb_in[0, 426:429] = gusum
    cpb_in[32, 429:432] = 1.0 / W1SC                           # s/32 bcast
    cpb_in[:, 432:560] = np.eye(128, dtype=np.float32)         # transpose id

    cpf_in = np.zeros((128, 8), dtype=np.float32)
    if has_b1e:
        for c in range(FCH):
            d, fh = c // 2, c % 2
            cpf_in[:, c] = b1e[d, fh * 128:(fh + 1) * 128]
    cpf_in[0:D, 6] = gb_eff
    cpf_in[:, 7] = 2.0 * EPS

    nc = _get_nc(has_b1e, has_b2)

    common = {
        "w1hi": w1hi_in,
        "w1lo": w1lo_in,
        "w2hi": w2hi_in,
        "w2lo": w2lo_in,
        "gq": gq_in,
        "cpb": _to_bf16(cpb_in),
        "cpf": cpf_in,
    }
    if has_b2:
        common["b2r"] = _to_bf16(b2)
    in_maps = [dict(common, xin=np.ascontiguousarray(x[c]))
               for c in range(B)]
    res = run_bass_kernel_spmd(nc, in_maps, core_ids=list(range(B)))
    global last_results
    last_results = res
    return np.stack([res.results[c]["out"] for c in range(B)])
